# revision 2
# baseline (speedup 1.0000x reference)
"""Self-contained Trainium2 Bass kernel for nn_Attention (8-head self-attention).

Reference computation (per batch element b):
    xt = x[b].reshape(C, N).T            # (N, C),  N = H*W = 1024
    q  = xt @ Wq                         # (N, 512)
    k, v = split(xt @ Wkv)               # (N, 512) each
    per head h (d=64): sim = q_h k_h^T / 8 ; P = softmax(sim) ; o_h = P v_h
    out[b] = concat_h(o_h) @ Wo + bo     # (N, C)

Sharding: pure data parallel -- core b computes batch element b (8 cores, 8
batch elements, no collectives).

Performance architecture (measured on HW):
  - Scalar exp throughput is the pacing floor: 64 exps of [128,1024]
    ~= 71.5us. The schedule keeps the exp stream as gapless as possible;
    Scalar runs exps ONLY until the tail (casts on DVE / GpSimd).
  - PE matmuls on DISJOINT row tiles execute concurrently (measured
    212ns/mm for alternating [64,128] tiles vs 588ns same-tile). Even/odd
    heads live on partitions 0:64 / 64:128 of qT/kT; sim matmuls are
    emitted head-alternating so the K=64 sim phase runs at ~full PE rate.
  - Head PAIRS are processed in 8 key-tile slots (2 exps each). attn@v,
    the remaining projections, and the early output-projection partials
    are placed into specific slots (AV_PLAN / CHUNKS) in ~4-matmul groups
    so per-slot PE work tracks the ~2.2us Scalar slot and every group's
    inputs (DMA arrivals, exp completions, PSUM frees) land just ahead.
  - Inputs load as big contiguous DMAs (x chunks, then full Wq / Wkv-k /
    Wkv-v / Wo row blocks, cast-sliced on GpSimd) plus two tiny strided
    column-tile DMAs so pair 0 gates only on x + 1/4 of Wq + 1/4 of Wkv.
  - Softmax denominators ride row 64 of the attn@v PSUM accumulator
    (ones column in v_ext). Reciprocals must run spread across 128
    partitions (serial ~6.4ns/elem per lane otherwise): DRAM bounce to
    [128,8], recip, bounce back for the 64-row broadcast. The two tail
    chains run on separate DMA queues with Scalar doing the PSUM row
    copies. Output projection for m=0..2 (+bias) is pre-accumulated into
    SBUF during pair 3, so after the final normalize each output tile is
    one matmul + one DVE add away from its DMA.
"""

import numpy as np

import concourse.bass as bass
import concourse.mybir as mybir
import concourse.tile as tile
from concourse import bacc

B, C, N = 8, 512, 1024
HEADS, D = 8, 64
INNER = HEADS * D  # 512
SCALE = D ** -0.5
P = 128
CT = C // P       # 4  k-tiles over C
MT = INNER // P   # 4  partition-tiles over inner
JT = N // P       # 8  key tiles
NT = N // P       # 8  output row tiles
NB = N // 512     # 2  free-dim blocks of 512 over N

F32 = mybir.dt.float32
F32R = mybir.dt.float32r
BF16 = mybir.dt.bfloat16
EXP = mybir.ActivationFunctionType.Exp

WARM_MM = 4       # dummy matmuls to touch the PE before real work
WARM_MM_GAP = 2   # dummies between kq0 accumulation steps (DMA-paced)

# (pair, slot) -> list of (src_pair, key_tile) attn@v steps to emit there.
AV_PLAN = {
    0: {3: [(0, 0)], 4: [(0, 1)], 5: [(0, 2)], 6: [(0, 3)],
        7: [(0, 4), (0, 5)]},
    1: {0: [(0, 6), (0, 7)], 2: [(1, 0)], 3: [(1, 1)], 4: [(1, 2)],
        5: [(1, 3)], 6: [(1, 4), (1, 5)], 7: [(1, 6)]},
    2: {0: [(1, 7)], 2: [(2, 0)], 3: [(2, 1)], 4: [(2, 2)],
        5: [(2, 3)], 6: [(2, 4), (2, 5)], 7: [(2, 6)]},
    3: {0: [(2, 7)], 2: [(3, 0)], 3: [(3, 1)], 4: [(3, 2)],
        5: [(3, 3)], 6: [(3, 4)], 7: [(3, 5)]},
}
# previous pair's denominator processing is staged across slots so the
# in-order DVE queue never blocks on DMA latency: PSUM copies + first two
# chain DMAs at slot 1, reciprocals + last two DMAs at slot 3, the
# normalize multiplies at slot 5.
DENOM_COPY_SLOT = 1
DENOM_RECIP_SLOT = 3
DENOM_MUL_SLOT = 5


def build_nc(debug=False):
    nc = bacc.Bacc(
        "TRN2", target_bir_lowering=False, debug=debug, num_devices=B
    )
    x_d = nc.dram_tensor("x", [C, N], F32, kind="ExternalInput")
    wq_d = nc.dram_tensor("Wq", [C, INNER], F32, kind="ExternalInput")
    wkv_d = nc.dram_tensor("Wkv", [C, 2 * INNER], F32, kind="ExternalInput")
    wo_d = nc.dram_tensor("Wo", [INNER, C], F32, kind="ExternalInput")
    bo_d = nc.dram_tensor("bo", [C], F32, kind="ExternalInput")
    out_d = nc.dram_tensor("out", [N, C], F32, kind="ExternalOutput")

    with tile.TileContext(nc) as tc:
        with (
            tc.tile_pool(name="persist", bufs=1) as persist,
            tc.tile_pool(name="stage", bufs=1) as stage,
            tc.tile_pool(name="etp", bufs=4) as etp,
            tc.tile_pool(name="small", bufs=2) as small,
            tc.tile_pool(name="dramp", bufs=2, space="DRAM") as dramp,
            tc.tile_pool(name="psS", bufs=2, space="PSUM") as psS,
            tc.tile_pool(name="psO", bufs=2, space="PSUM") as psO,
        ):
            # ---------------- constants + PE warmup ----------------
            zb = persist.tile([P, 1], F32)
            nc.vector.memset(zb, 0.0)
            zw = persist.tile([P, 512], BF16)
            nc.vector.memset(zw, 0.0)
            v_ext = persist.tile([P, JT, HEADS, D + 1], BF16)
            nc.vector.memset(v_ext[:, :, :, D], 1.0)

            warm = psO.tile([D + 1, N], F32, tag="ov")
            warm_i = [0]

            def warm_mm(k):
                for _ in range(k):
                    i = warm_i[0]
                    warm_i[0] += 1
                    nc.tensor.matmul(
                        warm[:, (i % 2) * 512:(i % 2) * 512 + 512],
                        lhsT=zw[:, 0:D + 1],
                        rhs=zw[:, 0:512],
                        start=True,
                        stop=True,
                    )

            warm_mm(WARM_MM)

            # ---------------- input DMA + casts (phase 1) --------------
            # kq0's gate: tiny strided column tiles of Wq / Wkv-k first,
            # then the x chunks (cast per chunk on DVE).
            wq0_f = stage.tile([P, CT, P], F32, tag="st_w0q")
            wq_b = persist.tile([P, MT, CT, P], BF16)
            wkvk0_f = stage.tile([P, CT, P], F32, tag="st_w0k")
            wkvk_b = persist.tile([P, MT, CT, P], BF16)
            nc.sync.dma_start(
                out=wq0_f, in_=wq_d[:, 0:P].rearrange("(a p) m -> p a m", p=P))
            nc.gpsimd.tensor_copy(
                out=wq_b[:, 0], in_=wq0_f)
            nc.sync.dma_start(
                out=wkvk0_f,
                in_=wkv_d[:, 0:P].rearrange("(a p) m -> p a m", p=P))
            nc.gpsimd.tensor_copy(
                out=wkvk_b[:, 0], in_=wkvk0_f)
            x_f = stage.tile([P, CT, N], F32, tag="st_x")
            x_b = persist.tile([P, CT, N], BF16)
            x_dv = x_d[:].rearrange("(a p) n -> p a n", p=P)
            for a in range(CT):
                nc.sync.dma_start(out=x_f[:, a, :], in_=x_dv[:, a, :])
                nc.vector.tensor_copy(out=x_b[:, a, :], in_=x_f[:, a, :])

            # persistent activation tiles: qT/kT (inner, N); head h lives at
            # partitions (h%2)*64, tile index h//2.
            qT = persist.tile([P, MT, N], BF16)
            kT = persist.tile([P, MT, N], BF16)
            oTs = []
            for m in range(MT):
                oTs.append(persist.tile([P, N], BF16, tag=f"oT{m}",
                                        name=f"oT{m}"))
            a_sb = persist.tile([P, NT, C], BF16)

            # ---------------- kq0: DMA-paced, warm-interleaved ----------
            st_k = psS.tile([P, N], F32, tag="st")
            st_q = psS.tile([P, N], F32, tag="st")
            for a in range(CT):
                for st, wb in ((st_k, wkvk_b), (st_q, wq_b)):
                    for ib in range(NB):
                        nc.tensor.matmul(
                            st[:, ib * 512:(ib + 1) * 512],
                            lhsT=wb[:, 0, a],
                            rhs=x_b[:, a, ib * 512:(ib + 1) * 512],
                            start=(a == 0),
                            stop=(a == CT - 1),
                        )
                warm_mm(WARM_MM_GAP)
            # readouts: qT on Scalar (idle until the first exp), kT on DVE
            # split in halves so the first sims unlock sooner.
            nc.scalar.copy(out=qT[:, 0, 0:512], in_=st_q[:, 0:512])
            nc.vector.tensor_copy(out=kT[:, 0, 0:512], in_=st_k[:, 0:512])
            nc.scalar.copy(out=qT[:, 0, 512:N], in_=st_q[:, 512:N])
            nc.vector.tensor_copy(out=kT[:, 0, 512:N], in_=st_k[:, 512:N])

            # ---------------- input DMA + casts (phase 2) --------------
            # wkv-v first (V groups run in the first attention slots); the
            # bulk Wq / Wkv-k / Wo DMAs are issued on the GpSimd queue
            # behind the wkv-v casts, so x + wkv-v get full DMA bandwidth
            # before the bulk weights start streaming.
            wkvv_b = persist.tile([P, CT, INNER], BF16)
            wkvv_f = stage.tile([P, CT, INNER], F32, tag="st_wv")
            nc.sync.dma_start(
                out=wkvv_f,
                in_=wkv_d[:, INNER:2 * INNER].rearrange("(a p) m -> p a m", p=P))
            # casts split across engines (concurrent Pool casts measured
            # ~1.9us each): Scalar is idle until the first exp, DVE takes
            # one, GpSimd keeps the last as the bulk-weight-DMA gate.
            nc.scalar.copy(out=wkvv_b[:, 0], in_=wkvv_f[:, 0])
            nc.scalar.copy(out=wkvv_b[:, 1], in_=wkvv_f[:, 1])
            nc.vector.tensor_copy(out=wkvv_b[:, 2], in_=wkvv_f[:, 2])
            nc.gpsimd.tensor_copy(out=wkvv_b[:, 3], in_=wkvv_f[:, 3])
            wq_f = stage.tile([P, CT, INNER], F32, tag="st_w")
            nc.gpsimd.dma_start(
                out=wq_f, in_=wq_d[:].rearrange("(a p) m -> p a m", p=P))
            for mt in range(1, MT):
                nc.gpsimd.tensor_copy(
                    out=wq_b[:, mt], in_=wq_f[:, :, mt * P:(mt + 1) * P])
            wkvk_f = stage.tile([P, CT, INNER], F32, tag="st_w")
            nc.gpsimd.dma_start(
                out=wkvk_f,
                in_=wkv_d[:, 0:INNER].rearrange("(a p) m -> p a m", p=P))
            for mt in range(1, MT):
                nc.gpsimd.tensor_copy(
                    out=wkvk_b[:, mt], in_=wkvk_f[:, :, mt * P:(mt + 1) * P])
            wo_b = persist.tile([P, MT, C], BF16)
            wo_f = stage.tile([P, CT, C], F32, tag="st_wo")
            nc.gpsimd.dma_start(
                out=wo_f,
                in_=wo_d[:].rearrange("(a p) m -> p a m", p=P))
            nc.gpsimd.tensor_copy(out=wo_b, in_=wo_f)

            bo_bc = persist.tile([P, C], F32)
            bo_ap = bo_d[:]
            nc.gpsimd.dma_start(
                out=bo_bc,
                in_=bass.AP(tensor=bo_ap.tensor, offset=bo_ap.offset,
                            ap=[[0, P], [1, C]]),
            )

            # ---------------- slot work groups (~4 matmuls each) --------
            def kq_group(dst, wb, mt, ib):
                ps = psS.tile([P, N], F32, tag="st", name=f"kq{mt}_{ib}")
                for a in range(CT):
                    nc.tensor.matmul(
                        ps[:, ib * 512:(ib + 1) * 512],
                        lhsT=wb[:, mt, a],
                        rhs=x_b[:, a, ib * 512:(ib + 1) * 512],
                        start=(a == 0),
                        stop=(a == CT - 1),
                    )
                nc.vector.tensor_copy(
                    out=dst[:, mt, ib * 512:(ib + 1) * 512],
                    in_=ps[:, ib * 512:(ib + 1) * 512])

            def v_group(jts):
                for jt in jts:
                    ps = psS.tile([P, N], F32, tag="st", name=f"v{jt}")
                    for a in range(CT):
                        nc.tensor.matmul(
                            ps[:, 0:512],
                            lhsT=x_b[:, a, jt * P:(jt + 1) * P],
                            rhs=wkvv_b[:, a, :],
                            start=(a == 0),
                            stop=(a == CT - 1),
                        )
                    nc.vector.tensor_copy(
                        out=v_ext[:, jt, :, 0:D],
                        in_=ps[:, 0:512].rearrange("p (h d) -> p h d", h=HEADS),
                    )

            def a_group(it):
                # output projection partial m=0..2 for row tile it,
                # accumulated (+bias) into SBUF; the tail adds only m=3.
                ps = psS.tile([P, N], F32, tag="st", name=f"a{it}")
                for kk in range(MT - 1):
                    nc.tensor.matmul(
                        ps[:, 0:C],
                        lhsT=oTs[kk][:, it * P:(it + 1) * P],
                        rhs=wo_b[:, kk, :],
                        start=(kk == 0),
                        stop=(kk == MT - 2),
                    )
                nc.vector.tensor_add(a_sb[:, it, :], ps[:, 0:C], bo_bc)

            # (pair, slot) -> work groups; PREFIX groups run before the
            # slot's sims (the pair-1 sims need kq1 complete).
            CHUNKS = {
                (0, 1): lambda: v_group((0, 1)),
                (0, 2): lambda: v_group((2, 3)),
                (0, 3): lambda: v_group((4, 5)),
                (0, 4): lambda: v_group((6, 7)),
                (0, 5): lambda: (kq_group(qT, wq_b, 1, 0),
                                 kq_group(qT, wq_b, 1, 1)),
                (0, 7): lambda: (kq_group(kT, wkvk_b, 1, 0),
                                 kq_group(kT, wkvk_b, 1, 1)),
                (1, 1): lambda: (kq_group(kT, wkvk_b, 2, 0),
                                 kq_group(qT, wq_b, 2, 0)),
                (1, 3): lambda: (kq_group(kT, wkvk_b, 2, 1),
                                 kq_group(qT, wq_b, 2, 1)),
                (2, 0): lambda: (kq_group(kT, wkvk_b, 3, 0),
                                 kq_group(qT, wq_b, 3, 0)),
                (2, 2): lambda: (kq_group(kT, wkvk_b, 3, 1),
                                 kq_group(qT, wq_b, 3, 1)),
                (3, 6): lambda: (a_group(0), a_group(1)),
            }
            PREFIX_CHUNKS = {}

            # ---------------- attention: head pairs ----------------
            ovs = {}
            ets = {}

            def sim_head(hm, jt, i):
                hp = i * D
                st = psS.tile([P, N], F32, tag="st", name=f"sim{hm}_{jt}_{i}")
                for ib in range(NB):
                    nc.tensor.matmul(
                        st[:, ib * 512:(ib + 1) * 512],
                        lhsT=kT[hp:hp + D, hm, jt * P:(jt + 1) * P],
                        rhs=qT[hp:hp + D, hm, ib * 512:(ib + 1) * 512],
                        start=True,
                        stop=True,
                    )
                nc.scalar.activation(
                    out=ets[2 * hm + i][:, jt, :], in_=st, func=EXP,
                    bias=zb, scale=SCALE)

            def sim_pair(hm, jt):
                sts = [psS.tile([P, N], F32, tag="st", name=f"sim{hm}_{jt}_{i}")
                       for i in range(2)]
                for ib in range(NB):
                    for i, hp in enumerate((0, 64)):
                        nc.tensor.matmul(
                            sts[i][:, ib * 512:(ib + 1) * 512],
                            lhsT=kT[hp:hp + D, hm, jt * P:(jt + 1) * P],
                            rhs=qT[hp:hp + D, hm, ib * 512:(ib + 1) * 512],
                            start=True,
                            stop=True,
                        )
                for i in range(2):
                    h = 2 * hm + i
                    nc.scalar.activation(
                        out=ets[h][:, jt, :], in_=sts[i], func=EXP,
                        bias=zb, scale=SCALE)

            def av_head(h, jt):
                ov = ovs[h]
                et = ets[h]
                for ib in range(NB):
                    nc.tensor.matmul(
                        ov[:, ib * 512:(ib + 1) * 512],
                        lhsT=v_ext[:, jt, h, :],
                        rhs=et[:, jt, ib * 512:(ib + 1) * 512],
                        start=(jt == 0),
                        stop=(jt == JT - 1),
                    )

            def av_pair(hm, jt):
                av_head(2 * hm, jt)
                av_head(2 * hm + 1, jt)

            def recip_chain(src_row, rep, dma_eng):
                # [1,N] denom row (SBUF) -> DRAM -> [128,8] -> recip ->
                # DRAM -> [D,N] broadcast
                sd = dramp.tile([N], F32, tag="sd")
                dma_eng.dma_start(out=sd, in_=src_row)
                s_sp = small.tile([P, NT], F32, tag="s_sp")
                dma_eng.dma_start(
                    out=s_sp, in_=sd.rearrange("(p k) -> p k", k=NT))
                r_sp = small.tile([P, NT], F32, tag="r_sp")
                nc.vector.reciprocal(r_sp, s_sp)
                rd = dramp.tile([N], F32, tag="rd")
                dma_eng.dma_start(
                    out=rd.rearrange("(p k) -> p k", k=NT), in_=r_sp)
                rd_ap = rd[:]
                dma_eng.dma_start(
                    out=rep,
                    in_=bass.AP(tensor=rd_ap.tensor, offset=rd_ap.offset,
                                ap=[[0, D], [1, N]]),
                )

            # staged denominator processing (see DENOM_*_SLOT comments)
            dstate = {}

            def denom_copy(h):
                # copy O'+denom out of PSUM (frees the ov bank) and start
                # the DRAM bounce of the denom row
                ov = ovs.pop(h)
                ov_sb = small.tile([D + 1, N], F32, tag="ovsb",
                                   name=f"ovsb{h}")
                nc.vector.tensor_copy(out=ov_sb, in_=ov)
                sd = dramp.tile([N], F32, tag="sd", name=f"sd{h}")
                nc.sync.dma_start(out=sd, in_=ov_sb[D:D + 1, :])
                s_sp = small.tile([P, NT], F32, tag="s_sp", name=f"ssp{h}")
                nc.sync.dma_start(
                    out=s_sp, in_=sd.rearrange("(p k) -> p k", k=NT))
                dstate[h] = (ov_sb, s_sp)

            def denom_recip(h):
                ov_sb, s_sp = dstate[h]
                r_sp = small.tile([P, NT], F32, tag="r_sp", name=f"rsp{h}")
                nc.vector.reciprocal(r_sp, s_sp)
                rd = dramp.tile([N], F32, tag="rd", name=f"rd{h}")
                nc.sync.dma_start(
                    out=rd.rearrange("(p k) -> p k", k=NT), in_=r_sp)
                rep = small.tile([D, N], F32, tag="rep", name=f"rep{h}")
                rd_ap = rd[:]
                nc.sync.dma_start(
                    out=rep,
                    in_=bass.AP(tensor=rd_ap.tensor, offset=rd_ap.offset,
                                ap=[[0, D], [1, N]]),
                )
                dstate[h] = (ov_sb, rep)

            def denom_mul(h, mul_eng=None):
                ov_sb, rep = dstate.pop(h)
                hp = (h % 2) * D
                eng = mul_eng or nc.vector
                eng.tensor_mul(oTs[h // 2][hp:hp + D, :], ov_sb[0:D, :], rep)
                del ets[h]

            for hm in range(MT):
                h0, h1 = 2 * hm, 2 * hm + 1
                ets[h0] = etp.tile([P, JT, N], BF16, tag="et", name=f"et{h0}")
                ets[h1] = etp.tile([P, JT, N], BF16, tag="et", name=f"et{h1}")
                plan = AV_PLAN[hm]
                for jt in range(JT):
                    sim_pair(hm, jt)
                    if hm > 0:
                        if jt == DENOM_COPY_SLOT:
                            denom_copy(h0 - 2)
                            denom_copy(h1 - 2)
                        elif jt == DENOM_RECIP_SLOT:
                            denom_recip(h0 - 2)
                            denom_recip(h1 - 2)
                        elif jt == DENOM_MUL_SLOT:
                            denom_mul(h0 - 2)
                            denom_mul(h1 - 2, mul_eng=nc.gpsimd)
                    if jt == 0:
                        ovs[h0] = psO.tile([D + 1, N], F32, tag="ov",
                                           name=f"ov{h0}")
                        ovs[h1] = psO.tile([D + 1, N], F32, tag="ov",
                                           name=f"ov{h1}")
                    for sp, kt in plan.get(jt, ()):
                        av_pair(sp, kt)
                    ch = CHUNKS.get((hm, jt))
                    if ch is not None:
                        ch()

            # ---------------- tail ----------------
            # Late normalization for the last pair: the reciprocal comes
            # back COLUMN-major ([128, 8]: partition q%128, column q//128)
            # after only 2 DMA hops, and is applied as a per-partition
            # Scalar scale on per-head output-projection partials -- no
            # 64-row broadcast needed. a-groups keep the PE warm.
            h6, h7 = HEADS - 2, HEADS - 1
            av_pair(3, JT - 2)
            av_head(h6, JT - 1)
            row6_t = small.tile([D, N], F32, tag="rep", name="row6")
            row6 = row6_t[0:1, :]
            nc.scalar.copy(out=row6, in_=ovs[h6][D:D + 1, :])
            sd6 = dramp.tile([N], F32, tag="sd", name="sd6")
            nc.sync.dma_start(out=sd6, in_=row6)
            s6c = small.tile([P, NT], F32, tag="s_sp", name="s6c")
            nc.sync.dma_start(
                out=s6c, in_=sd6.rearrange("(k p) -> p k", p=P))
            av_head(h7, JT - 1)
            row7_t = small.tile([D, N], F32, tag="rep", name="row7")
            row7 = row7_t[0:1, :]
            nc.scalar.copy(out=row7, in_=ovs[h7][D:D + 1, :])
            sd7 = dramp.tile([N], F32, tag="sd", name="sd7")
            nc.sync.dma_start(out=sd7, in_=row7)
            s7c = small.tile([P, NT], F32, tag="s_sp", name="s7c")
            nc.sync.dma_start(
                out=s7c, in_=sd7.rearrange("(k p) -> p k", p=P))

            for it in range(2, NT):
                a_group(it)

            # unnormalized O' for both heads (lhsT of the B matmuls)
            oT3u = persist.tile([P, N], BF16)
            ov6 = ovs.pop(h6)
            nc.vector.tensor_copy(out=oT3u[0:D, :], in_=ov6[0:D, :])
            ov7 = ovs.pop(h7)
            nc.vector.tensor_copy(out=oT3u[D:P, :], in_=ov7[0:D, :])
            r6c = small.tile([P, NT], F32, tag="r_sp", name="r6c")
            nc.vector.reciprocal(r6c, s6c)
            r7c = small.tile([P, NT], F32, tag="r_sp", name="r7c")
            nc.vector.reciprocal(r7c, s7c)
            del ets[h6], ets[h7]

            # per output tile: two K=64 head-partials (alternating PE row
            # tiles), Scalar applies 1/denom per partition, DVE folds in
            # the m=0..2+bias partial
            for it in range(NT):
                pb = psS.tile([P, N], F32, tag="st", name=f"pb{it}")
                for half, hp in ((0, 0), (1, D)):
                    nc.tensor.matmul(
                        pb[:, half * 512:half * 512 + C],
                        lhsT=oT3u[hp:hp + D, it * P:(it + 1) * P],
                        rhs=wo_b[hp:hp + D, 3, :],
                        start=True,
                        stop=True,
                    )
                t6 = small.tile([P, C], F32, tag="t6", name=f"t6_{it}")
                nc.scalar.activation(
                    out=t6, in_=pb[:, 0:C],
                    func=mybir.ActivationFunctionType.Copy,
                    scale=r6c[:, it:it + 1])
                t67 = small.tile([P, C], F32, tag="t7", name=f"t67_{it}")
                nc.vector.scalar_tensor_tensor(
                    out=t67, in0=pb[:, 512:512 + C],
                    scalar=r7c[:, it:it + 1], in1=t6,
                    op0=mybir.AluOpType.mult, op1=mybir.AluOpType.add)
                fin = small.tile([P, C], F32, tag="fin", bufs=3)
                eng = nc.vector if it % 2 == 0 else nc.gpsimd
                eng.tensor_add(fin, t67, a_sb[:, it, :])
                nc.sync.dma_start(out=out_d[it * P:(it + 1) * P, :], in_=fin)

    return nc


def kernel(x, Wq, Wkv, Wo, bo):
    from concourse.bass_utils import run_bass_kernel_spmd

    nc = build_nc()
    nc.compile()
    x = np.asarray(x)
    xs = np.ascontiguousarray(x.reshape(B, C, N)).astype(np.float32, copy=False)
    in_maps = [
        {
            "x": xs[b],
            "Wq": np.asarray(Wq, dtype=np.float32),
            "Wkv": np.asarray(Wkv, dtype=np.float32),
            "Wo": np.asarray(Wo, dtype=np.float32),
            "bo": np.asarray(bo, dtype=np.float32),
        }
        for b in range(B)
    ]
    res = run_bass_kernel_spmd(nc, in_maps, list(range(B)))
    return np.stack([res.results[b]["out"] for b in range(B)], axis=0)


# revision 3
# speedup vs baseline: 1.1829x; 1.1829x over previous
"""Self-contained Trainium2 Bass kernel for nn_Attention (8-head self-attention).

Reference computation (per batch element b):
    xt = x[b].reshape(C, N).T            # (N, C),  N = H*W = 1024
    q  = xt @ Wq                         # (N, 512)
    k, v = split(xt @ Wkv)               # (N, 512) each
    per head h (d=64): sim = q_h k_h^T / 8 ; P = softmax(sim) ; o_h = P v_h
    out[b] = concat_h(o_h) @ Wo + bo     # (N, C)

Sharding: pure data parallel -- core b computes batch element b (8 cores, 8
batch elements, no collectives).

Performance architecture (measured on HW):
  - Scalar exp throughput is the pacing floor: 64 exps of [128,1024]
    ~= 71.5us. The schedule keeps the exp stream as gapless as possible;
    Scalar runs exps ONLY until the tail (casts on DVE / GpSimd).
  - PE matmuls on DISJOINT row tiles execute concurrently (measured
    212ns/mm for alternating [64,128] tiles vs 588ns same-tile). Even/odd
    heads live on partitions 0:64 / 64:128 of qT/kT; sim matmuls are
    emitted head-alternating so the K=64 sim phase runs at ~full PE rate.
  - Head PAIRS are processed in 8 key-tile slots (2 exps each). attn@v,
    the remaining projections, and the early output-projection partials
    are placed into specific slots (AV_PLAN / CHUNKS) in ~4-matmul groups
    so per-slot PE work tracks the ~2.2us Scalar slot and every group's
    inputs (DMA arrivals, exp completions, PSUM frees) land just ahead.
  - Inputs load as big contiguous DMAs (x chunks, then full Wq / Wkv-k /
    Wkv-v / Wo row blocks, cast-sliced on GpSimd) plus two tiny strided
    column-tile DMAs so pair 0 gates only on x + 1/4 of Wq + 1/4 of Wkv.
  - Softmax denominators ride row 64 of the attn@v PSUM accumulator
    (ones column in v_ext). Reciprocals must run spread across 128
    partitions (serial ~6.4ns/elem per lane otherwise): DRAM bounce to
    [128,8], recip, bounce back for the 64-row broadcast. The two tail
    chains run on separate DMA queues with Scalar doing the PSUM row
    copies. Output projection for m=0..2 (+bias) is pre-accumulated into
    SBUF during pair 3, so after the final normalize each output tile is
    one matmul + one DVE add away from its DMA.
"""

import numpy as np

import concourse.bass as bass
import concourse.mybir as mybir
import concourse.tile as tile
from concourse import bacc

B, C, N = 8, 512, 1024
HEADS, D = 8, 64
INNER = HEADS * D  # 512
SCALE = D ** -0.5
P = 128
CT = C // P       # 4  k-tiles over C
MT = INNER // P   # 4  partition-tiles over inner
JT = N // P       # 8  key tiles
NT = N // P       # 8  output row tiles
NB = N // 512     # 2  free-dim blocks of 512 over N

F32 = mybir.dt.float32
F32R = mybir.dt.float32r
BF16 = mybir.dt.bfloat16
EXP = mybir.ActivationFunctionType.Exp

WARM_MM = 4       # dummy matmuls to touch the PE before real work
WARM_MM_GAP = 2   # dummies between kq0 accumulation steps (DMA-paced)

# (pair, slot) -> list of (src_pair, key_tile) attn@v steps to emit there.
AV_PLAN = {
    0: {3: [(0, 0)], 4: [(0, 1)], 5: [(0, 2)], 6: [(0, 3)],
        7: [(0, 4), (0, 5)]},
    1: {0: [(0, 6), (0, 7)], 2: [(1, 0)], 3: [(1, 1)], 4: [(1, 2)],
        5: [(1, 3)], 6: [(1, 4), (1, 5)], 7: [(1, 6)]},
    2: {0: [(1, 7)], 2: [(2, 0)], 3: [(2, 1)], 4: [(2, 2)],
        5: [(2, 3)], 6: [(2, 4), (2, 5)], 7: [(2, 6)]},
    3: {0: [(2, 7)], 2: [(3, 0)], 3: [(3, 1)], 4: [(3, 2)],
        5: [(3, 3)], 6: [(3, 4)], 7: [(3, 5)]},
}
# previous pair's denominator processing is staged across slots so the
# in-order DVE queue never blocks on DMA latency: PSUM copies + first two
# chain DMAs at slot 1, reciprocals + last two DMAs at slot 3, the
# normalize multiplies at slot 5.
DENOM_COPY_SLOT = 1
DENOM_RECIP_SLOT = 3
DENOM_MUL_SLOT = 5


def build_nc(debug=False):
    nc = bacc.Bacc(
        "TRN2", target_bir_lowering=False, debug=debug, num_devices=B
    )
    x_d = nc.dram_tensor("x", [C, N], F32, kind="ExternalInput")
    wq_d = nc.dram_tensor("Wq", [C, INNER], F32, kind="ExternalInput")
    wkv_d = nc.dram_tensor("Wkv", [C, 2 * INNER], F32, kind="ExternalInput")
    wo_d = nc.dram_tensor("Wo", [INNER, C], F32, kind="ExternalInput")
    bo_d = nc.dram_tensor("bo", [C], F32, kind="ExternalInput")
    out_d = nc.dram_tensor("out", [N, C], F32, kind="ExternalOutput")

    with tile.TileContext(nc) as tc:
        with (
            tc.tile_pool(name="persist", bufs=1) as persist,
            tc.tile_pool(name="stage", bufs=1) as stage,
            tc.tile_pool(name="etp", bufs=4) as etp,
            tc.tile_pool(name="small", bufs=2) as small,
            tc.tile_pool(name="dramp", bufs=2, space="DRAM") as dramp,
            tc.tile_pool(name="psS", bufs=2, space="PSUM") as psS,
            tc.tile_pool(name="psO", bufs=2, space="PSUM") as psO,
        ):
            # ---------------- constants + PE warmup ----------------
            zb = persist.tile([P, 1], F32)
            nc.vector.memset(zb, 0.0)
            zw = persist.tile([P, 512], BF16)
            nc.vector.memset(zw, 0.0)
            v_ext = persist.tile([P, JT, HEADS, D + 1], BF16)
            nc.vector.memset(v_ext[:, :, :, D], 1.0)

            warm = psO.tile([D + 1, N], F32, tag="ov")
            warm_i = [0]

            def warm_mm(k):
                for _ in range(k):
                    i = warm_i[0]
                    warm_i[0] += 1
                    nc.tensor.matmul(
                        warm[:, (i % 2) * 512:(i % 2) * 512 + 512],
                        lhsT=zw[:, 0:D + 1],
                        rhs=zw[:, 0:512],
                        start=True,
                        stop=True,
                    )

            warm_mm(WARM_MM)

            # ---------------- input DMA + casts (phase 1) --------------
            # kq0's gate: tiny strided column tiles of Wq / Wkv-k first,
            # then the x chunks (cast per chunk on DVE).
            wq0_f = stage.tile([P, CT, P], F32, tag="st_w0q")
            wq_b = persist.tile([P, MT, CT, P], BF16)
            wkvk0_f = stage.tile([P, CT, P], F32, tag="st_w0k")
            wkvk_b = persist.tile([P, MT, CT, P], BF16)
            nc.sync.dma_start(
                out=wq0_f, in_=wq_d[:, 0:P].rearrange("(a p) m -> p a m", p=P))
            nc.gpsimd.tensor_copy(
                out=wq_b[:, 0], in_=wq0_f)
            nc.sync.dma_start(
                out=wkvk0_f,
                in_=wkv_d[:, 0:P].rearrange("(a p) m -> p a m", p=P))
            nc.gpsimd.tensor_copy(
                out=wkvk_b[:, 0], in_=wkvk0_f)
            x_f = stage.tile([P, CT, N], F32, tag="st_x")
            x_b = persist.tile([P, CT, N], BF16)
            x_dv = x_d[:].rearrange("(a p) n -> p a n", p=P)
            for a in range(CT):
                nc.sync.dma_start(out=x_f[:, a, :], in_=x_dv[:, a, :])
                nc.vector.tensor_copy(out=x_b[:, a, :], in_=x_f[:, a, :])

            # persistent activation tiles: qT/kT (inner, N); head h lives at
            # partitions (h%2)*64, tile index h//2.
            qT = persist.tile([P, MT, N], BF16)
            kT = persist.tile([P, MT, N], BF16)
            oTs = []
            for m in range(MT):
                oTs.append(persist.tile([P, N], BF16, tag=f"oT{m}",
                                        name=f"oT{m}"))
            a_sb = persist.tile([P, NT, C], BF16)

            # ---------------- kq0: DMA-paced, warm-interleaved ----------
            st_k = psS.tile([P, N], F32, tag="st")
            st_q = psS.tile([P, N], F32, tag="st")
            for a in range(CT):
                for st, wb in ((st_k, wkvk_b), (st_q, wq_b)):
                    for ib in range(NB):
                        nc.tensor.matmul(
                            st[:, ib * 512:(ib + 1) * 512],
                            lhsT=wb[:, 0, a],
                            rhs=x_b[:, a, ib * 512:(ib + 1) * 512],
                            start=(a == 0),
                            stop=(a == CT - 1),
                        )
                warm_mm(WARM_MM_GAP)
            # readouts: qT on Scalar (idle until the first exp), kT on DVE
            # split in halves so the first sims unlock sooner.
            nc.scalar.copy(out=qT[:, 0, 0:512], in_=st_q[:, 0:512])
            nc.vector.tensor_copy(out=kT[:, 0, 0:512], in_=st_k[:, 0:512])
            nc.scalar.copy(out=qT[:, 0, 512:N], in_=st_q[:, 512:N])
            nc.vector.tensor_copy(out=kT[:, 0, 512:N], in_=st_k[:, 512:N])

            # ---------------- input DMA + casts (phase 2) --------------
            # wkv-v first (V groups run in the first attention slots); the
            # bulk Wq / Wkv-k / Wo DMAs are issued on the GpSimd queue
            # behind the wkv-v casts, so x + wkv-v get full DMA bandwidth
            # before the bulk weights start streaming.
            wkvv_b = persist.tile([P, CT, INNER], BF16)
            wkvv_f = stage.tile([P, CT, INNER], F32, tag="st_wv")
            nc.sync.dma_start(
                out=wkvv_f,
                in_=wkv_d[:, INNER:2 * INNER].rearrange("(a p) m -> p a m", p=P))
            # casts split across engines (concurrent Pool casts measured
            # ~1.9us each): Scalar is idle until the first exp, DVE takes
            # one, GpSimd keeps the last as the bulk-weight-DMA gate.
            # none on Scalar: anything here would sit between the qT
            # readouts and the first exp in the in-order Scalar queue,
            # delaying the whole exp stream by the wkv-v DMA wait
            nc.gpsimd.tensor_copy(out=wkvv_b[:, 0], in_=wkvv_f[:, 0])
            nc.vector.tensor_copy(out=wkvv_b[:, 1], in_=wkvv_f[:, 1])
            nc.vector.tensor_copy(out=wkvv_b[:, 2], in_=wkvv_f[:, 2])
            nc.gpsimd.tensor_copy(out=wkvv_b[:, 3], in_=wkvv_f[:, 3])
            wq_f = stage.tile([P, CT, INNER], F32, tag="st_w")
            nc.gpsimd.dma_start(
                out=wq_f, in_=wq_d[:].rearrange("(a p) m -> p a m", p=P))
            for mt in range(1, MT):
                nc.gpsimd.tensor_copy(
                    out=wq_b[:, mt], in_=wq_f[:, :, mt * P:(mt + 1) * P])
            wkvk_f = stage.tile([P, CT, INNER], F32, tag="st_w")
            nc.gpsimd.dma_start(
                out=wkvk_f,
                in_=wkv_d[:, 0:INNER].rearrange("(a p) m -> p a m", p=P))
            for mt in range(1, MT):
                nc.gpsimd.tensor_copy(
                    out=wkvk_b[:, mt], in_=wkvk_f[:, :, mt * P:(mt + 1) * P])
            wo_b = persist.tile([P, MT, C], BF16)
            wo_f = stage.tile([P, CT, C], F32, tag="st_wo")
            nc.gpsimd.dma_start(
                out=wo_f,
                in_=wo_d[:].rearrange("(a p) m -> p a m", p=P))
            nc.gpsimd.tensor_copy(out=wo_b, in_=wo_f)

            bo_bc = persist.tile([P, C], F32)
            bo_ap = bo_d[:]
            nc.gpsimd.dma_start(
                out=bo_bc,
                in_=bass.AP(tensor=bo_ap.tensor, offset=bo_ap.offset,
                            ap=[[0, P], [1, C]]),
            )

            # ---------------- slot work groups (~4 matmuls each) --------
            def kq_group(dst, wb, mt, ib):
                ps = psS.tile([P, N], F32, tag="st", name=f"kq{mt}_{ib}")
                for a in range(CT):
                    nc.tensor.matmul(
                        ps[:, ib * 512:(ib + 1) * 512],
                        lhsT=wb[:, mt, a],
                        rhs=x_b[:, a, ib * 512:(ib + 1) * 512],
                        start=(a == 0),
                        stop=(a == CT - 1),
                    )
                nc.vector.tensor_copy(
                    out=dst[:, mt, ib * 512:(ib + 1) * 512],
                    in_=ps[:, ib * 512:(ib + 1) * 512])

            def v_group(jts):
                for jt in jts:
                    ps = psS.tile([P, N], F32, tag="st", name=f"v{jt}")
                    for a in range(CT):
                        nc.tensor.matmul(
                            ps[:, 0:512],
                            lhsT=x_b[:, a, jt * P:(jt + 1) * P],
                            rhs=wkvv_b[:, a, :],
                            start=(a == 0),
                            stop=(a == CT - 1),
                        )
                    nc.vector.tensor_copy(
                        out=v_ext[:, jt, :, 0:D],
                        in_=ps[:, 0:512].rearrange("p (h d) -> p h d", h=HEADS),
                    )

            def a_group(it):
                # output projection partial m=0..2 for row tile it,
                # accumulated (+bias) into SBUF; the tail adds only m=3.
                ps = psS.tile([P, N], F32, tag="st", name=f"a{it}")
                for kk in range(MT - 1):
                    nc.tensor.matmul(
                        ps[:, 0:C],
                        lhsT=oTs[kk][:, it * P:(it + 1) * P],
                        rhs=wo_b[:, kk, :],
                        start=(kk == 0),
                        stop=(kk == MT - 2),
                    )
                nc.vector.tensor_add(a_sb[:, it, :], ps[:, 0:C], bo_bc)

            # (pair, slot) -> work groups; PREFIX groups run before the
            # slot's sims (the pair-1 sims need kq1 complete).
            CHUNKS = {
                (0, 1): lambda: v_group((0, 1)),
                (0, 2): lambda: v_group((2, 3)),
                (0, 3): lambda: v_group((4, 5)),
                (0, 4): lambda: v_group((6, 7)),
                (0, 5): lambda: (kq_group(qT, wq_b, 1, 0),
                                 kq_group(qT, wq_b, 1, 1)),
                (0, 7): lambda: (kq_group(kT, wkvk_b, 1, 0),
                                 kq_group(kT, wkvk_b, 1, 1)),
                (1, 1): lambda: (kq_group(kT, wkvk_b, 2, 0),
                                 kq_group(qT, wq_b, 2, 0)),
                (1, 3): lambda: (kq_group(kT, wkvk_b, 2, 1),
                                 kq_group(qT, wq_b, 2, 1)),
                (2, 0): lambda: (kq_group(kT, wkvk_b, 3, 0),
                                 kq_group(qT, wq_b, 3, 0)),
                (2, 2): lambda: (kq_group(kT, wkvk_b, 3, 1),
                                 kq_group(qT, wq_b, 3, 1)),
                (3, 6): lambda: (a_group(0), a_group(1)),
            }
            PREFIX_CHUNKS = {}

            # ---------------- attention: head pairs ----------------
            ovs = {}
            ets = {}

            def sim_head(hm, jt, i):
                hp = i * D
                st = psS.tile([P, N], F32, tag="st", name=f"sim{hm}_{jt}_{i}")
                for ib in range(NB):
                    nc.tensor.matmul(
                        st[:, ib * 512:(ib + 1) * 512],
                        lhsT=kT[hp:hp + D, hm, jt * P:(jt + 1) * P],
                        rhs=qT[hp:hp + D, hm, ib * 512:(ib + 1) * 512],
                        start=True,
                        stop=True,
                    )
                nc.scalar.activation(
                    out=ets[2 * hm + i][:, jt, :], in_=st, func=EXP,
                    bias=zb, scale=SCALE)

            def sim_pair(hm, jt):
                sts = [psS.tile([P, N], F32, tag="st", name=f"sim{hm}_{jt}_{i}")
                       for i in range(2)]
                for ib in range(NB):
                    for i, hp in enumerate((0, 64)):
                        nc.tensor.matmul(
                            sts[i][:, ib * 512:(ib + 1) * 512],
                            lhsT=kT[hp:hp + D, hm, jt * P:(jt + 1) * P],
                            rhs=qT[hp:hp + D, hm, ib * 512:(ib + 1) * 512],
                            start=True,
                            stop=True,
                        )
                for i in range(2):
                    h = 2 * hm + i
                    nc.scalar.activation(
                        out=ets[h][:, jt, :], in_=sts[i], func=EXP,
                        bias=zb, scale=SCALE)

            def av_head(h, jt):
                ov = ovs[h]
                et = ets[h]
                for ib in range(NB):
                    nc.tensor.matmul(
                        ov[:, ib * 512:(ib + 1) * 512],
                        lhsT=v_ext[:, jt, h, :],
                        rhs=et[:, jt, ib * 512:(ib + 1) * 512],
                        start=(jt == 0),
                        stop=(jt == JT - 1),
                    )

            def av_pair(hm, jt):
                av_head(2 * hm, jt)
                av_head(2 * hm + 1, jt)

            def recip_chain(src_row, rep, dma_eng):
                # [1,N] denom row (SBUF) -> DRAM -> [128,8] -> recip ->
                # DRAM -> [D,N] broadcast
                sd = dramp.tile([N], F32, tag="sd")
                dma_eng.dma_start(out=sd, in_=src_row)
                s_sp = small.tile([P, NT], F32, tag="s_sp")
                dma_eng.dma_start(
                    out=s_sp, in_=sd.rearrange("(p k) -> p k", k=NT))
                r_sp = small.tile([P, NT], F32, tag="r_sp")
                nc.vector.reciprocal(r_sp, s_sp)
                rd = dramp.tile([N], F32, tag="rd")
                dma_eng.dma_start(
                    out=rd.rearrange("(p k) -> p k", k=NT), in_=r_sp)
                rd_ap = rd[:]
                dma_eng.dma_start(
                    out=rep,
                    in_=bass.AP(tensor=rd_ap.tensor, offset=rd_ap.offset,
                                ap=[[0, D], [1, N]]),
                )

            # staged denominator processing (see DENOM_*_SLOT comments)
            dstate = {}

            def denom_copy(h):
                # copy O'+denom out of PSUM (frees the ov bank) and start
                # the DRAM bounce of the denom row
                ov = ovs.pop(h)
                ov_sb = small.tile([D + 1, N], F32, tag="ovsb",
                                   name=f"ovsb{h}")
                nc.vector.tensor_copy(out=ov_sb, in_=ov)
                sd = dramp.tile([N], F32, tag="sd", name=f"sd{h}")
                nc.sync.dma_start(out=sd, in_=ov_sb[D:D + 1, :])
                s_sp = small.tile([P, NT], F32, tag="s_sp", name=f"ssp{h}")
                nc.sync.dma_start(
                    out=s_sp, in_=sd.rearrange("(p k) -> p k", k=NT))
                dstate[h] = (ov_sb, s_sp)

            def denom_recip(h):
                ov_sb, s_sp = dstate[h]
                r_sp = small.tile([P, NT], F32, tag="r_sp", name=f"rsp{h}")
                nc.vector.reciprocal(r_sp, s_sp)
                rd = dramp.tile([N], F32, tag="rd", name=f"rd{h}")
                nc.sync.dma_start(
                    out=rd.rearrange("(p k) -> p k", k=NT), in_=r_sp)
                rep = small.tile([D, N], F32, tag="rep", name=f"rep{h}")
                rd_ap = rd[:]
                nc.sync.dma_start(
                    out=rep,
                    in_=bass.AP(tensor=rd_ap.tensor, offset=rd_ap.offset,
                                ap=[[0, D], [1, N]]),
                )
                dstate[h] = (ov_sb, rep)

            def denom_mul(h, mul_eng=None):
                ov_sb, rep = dstate.pop(h)
                hp = (h % 2) * D
                eng = mul_eng or nc.vector
                eng.tensor_mul(oTs[h // 2][hp:hp + D, :], ov_sb[0:D, :], rep)
                del ets[h]

            for hm in range(MT):
                h0, h1 = 2 * hm, 2 * hm + 1
                ets[h0] = etp.tile([P, JT, N], BF16, tag="et", name=f"et{h0}")
                ets[h1] = etp.tile([P, JT, N], BF16, tag="et", name=f"et{h1}")
                plan = AV_PLAN[hm]
                for jt in range(JT):
                    sim_pair(hm, jt)
                    if hm > 0:
                        if jt == DENOM_COPY_SLOT:
                            denom_copy(h0 - 2)
                            denom_copy(h1 - 2)
                        elif jt == DENOM_RECIP_SLOT:
                            denom_recip(h0 - 2)
                            denom_recip(h1 - 2)
                        elif jt == DENOM_MUL_SLOT:
                            denom_mul(h0 - 2)
                            denom_mul(h1 - 2, mul_eng=nc.gpsimd)
                    if jt == 0:
                        ovs[h0] = psO.tile([D + 1, N], F32, tag="ov",
                                           name=f"ov{h0}")
                        ovs[h1] = psO.tile([D + 1, N], F32, tag="ov",
                                           name=f"ov{h1}")
                    for sp, kt in plan.get(jt, ()):
                        av_pair(sp, kt)
                    ch = CHUNKS.get((hm, jt))
                    if ch is not None:
                        ch()

            # ---------------- tail ----------------
            # Late normalization for the last pair: the reciprocal comes
            # back COLUMN-major ([128, 8]: partition q%128, column q//128)
            # after only 2 DMA hops, and is applied as a per-partition
            # Scalar scale on per-head output-projection partials -- no
            # 64-row broadcast needed. a-groups keep the PE warm.
            h6, h7 = HEADS - 2, HEADS - 1
            av_pair(3, JT - 2)
            av_head(h6, JT - 1)
            row6_t = small.tile([D, N], F32, tag="rep", name="row6")
            row6 = row6_t[0:1, :]
            nc.scalar.copy(out=row6, in_=ovs[h6][D:D + 1, :])
            sd6 = dramp.tile([N], F32, tag="sd", name="sd6")
            nc.sync.dma_start(out=sd6, in_=row6)
            s6c = small.tile([P, NT], F32, tag="s_sp", name="s6c")
            nc.sync.dma_start(
                out=s6c, in_=sd6.rearrange("(k p) -> p k", p=P))
            av_head(h7, JT - 1)
            row7_t = small.tile([D, N], F32, tag="rep", name="row7")
            row7 = row7_t[0:1, :]
            nc.scalar.copy(out=row7, in_=ovs[h7][D:D + 1, :])
            sd7 = dramp.tile([N], F32, tag="sd", name="sd7")
            nc.sync.dma_start(out=sd7, in_=row7)
            s7c = small.tile([P, NT], F32, tag="s_sp", name="s7c")
            nc.sync.dma_start(
                out=s7c, in_=sd7.rearrange("(k p) -> p k", p=P))

            for it in range(2, NT):
                a_group(it)

            # unnormalized O' for both heads (lhsT of the B matmuls)
            oT3u = persist.tile([P, N], BF16)
            ov6 = ovs.pop(h6)
            nc.vector.tensor_copy(out=oT3u[0:D, :], in_=ov6[0:D, :])
            ov7 = ovs.pop(h7)
            nc.vector.tensor_copy(out=oT3u[D:P, :], in_=ov7[0:D, :])
            r6c = small.tile([P, NT], F32, tag="r_sp", name="r6c")
            nc.vector.reciprocal(r6c, s6c)
            r7c = small.tile([P, NT], F32, tag="r_sp", name="r7c")
            nc.vector.reciprocal(r7c, s7c)
            del ets[h6], ets[h7]

            # per output tile: two K=64 head-partials (alternating PE row
            # tiles), Scalar applies 1/denom per partition, DVE folds in
            # the m=0..2+bias partial
            for it in range(NT):
                pb = psS.tile([P, N], F32, tag="st", name=f"pb{it}")
                for half, hp in ((0, 0), (1, D)):
                    nc.tensor.matmul(
                        pb[:, half * 512:half * 512 + C],
                        lhsT=oT3u[hp:hp + D, it * P:(it + 1) * P],
                        rhs=wo_b[hp:hp + D, 3, :],
                        start=True,
                        stop=True,
                    )
                t6 = small.tile([P, C], F32, tag="t6", name=f"t6_{it}")
                nc.scalar.activation(
                    out=t6, in_=pb[:, 0:C],
                    func=mybir.ActivationFunctionType.Copy,
                    scale=r6c[:, it:it + 1])
                t67 = small.tile([P, C], F32, tag="t7", name=f"t67_{it}")
                nc.vector.scalar_tensor_tensor(
                    out=t67, in0=pb[:, 512:512 + C],
                    scalar=r7c[:, it:it + 1], in1=t6,
                    op0=mybir.AluOpType.mult, op1=mybir.AluOpType.add)
                fin = small.tile([P, C], F32, tag="fin", bufs=3)
                eng = nc.vector if it % 2 == 0 else nc.gpsimd
                eng.tensor_add(fin, t67, a_sb[:, it, :])
                nc.sync.dma_start(out=out_d[it * P:(it + 1) * P, :], in_=fin)

    return nc


def kernel(x, Wq, Wkv, Wo, bo):
    from concourse.bass_utils import run_bass_kernel_spmd

    nc = build_nc()
    nc.compile()
    x = np.asarray(x)
    xs = np.ascontiguousarray(x.reshape(B, C, N)).astype(np.float32, copy=False)
    in_maps = [
        {
            "x": xs[b],
            "Wq": np.asarray(Wq, dtype=np.float32),
            "Wkv": np.asarray(Wkv, dtype=np.float32),
            "Wo": np.asarray(Wo, dtype=np.float32),
            "bo": np.asarray(bo, dtype=np.float32),
        }
        for b in range(B)
    ]
    res = run_bass_kernel_spmd(nc, in_maps, list(range(B)))
    return np.stack([res.results[b]["out"] for b in range(B)], axis=0)


# revision 4
# speedup vs baseline: 1.1916x; 1.0073x over previous
"""Self-contained Trainium2 Bass kernel for nn_Attention (8-head self-attention).

Reference computation (per batch element b):
    xt = x[b].reshape(C, N).T            # (N, C),  N = H*W = 1024
    q  = xt @ Wq                         # (N, 512)
    k, v = split(xt @ Wkv)               # (N, 512) each
    per head h (d=64): sim = q_h k_h^T / 8 ; P = softmax(sim) ; o_h = P v_h
    out[b] = concat_h(o_h) @ Wo + bo     # (N, C)

Sharding: pure data parallel -- core b computes batch element b (8 cores, 8
batch elements, no collectives).

Performance architecture (measured on HW):
  - Scalar exp throughput is the pacing floor: 64 exps of [128,1024]
    ~= 71.5us. The schedule keeps the exp stream as gapless as possible;
    Scalar runs exps ONLY until the tail (casts on DVE / GpSimd).
  - PE matmuls on DISJOINT row tiles execute concurrently (measured
    212ns/mm for alternating [64,128] tiles vs 588ns same-tile). Even/odd
    heads live on partitions 0:64 / 64:128 of qT/kT; sim matmuls are
    emitted head-alternating so the K=64 sim phase runs at ~full PE rate.
  - Head PAIRS are processed in 8 key-tile slots (2 exps each). attn@v,
    the remaining projections, and the early output-projection partials
    are placed into specific slots (AV_PLAN / CHUNKS) in ~4-matmul groups
    so per-slot PE work tracks the ~2.2us Scalar slot and every group's
    inputs (DMA arrivals, exp completions, PSUM frees) land just ahead.
  - Inputs load as big contiguous DMAs (x chunks, then full Wq / Wkv-k /
    Wkv-v / Wo row blocks, cast-sliced on GpSimd) plus two tiny strided
    column-tile DMAs so pair 0 gates only on x + 1/4 of Wq + 1/4 of Wkv.
  - Softmax denominators ride row 64 of the attn@v PSUM accumulator
    (ones column in v_ext). Reciprocals must run spread across 128
    partitions (serial ~6.4ns/elem per lane otherwise): DRAM bounce to
    [128,8], recip, bounce back for the 64-row broadcast. The two tail
    chains run on separate DMA queues with Scalar doing the PSUM row
    copies. Output projection for m=0..2 (+bias) is pre-accumulated into
    SBUF during pair 3, so after the final normalize each output tile is
    one matmul + one DVE add away from its DMA.
"""

import numpy as np

import concourse.bass as bass
import concourse.mybir as mybir
import concourse.tile as tile
from concourse import bacc

B, C, N = 8, 512, 1024
HEADS, D = 8, 64
INNER = HEADS * D  # 512
SCALE = D ** -0.5
P = 128
CT = C // P       # 4  k-tiles over C
MT = INNER // P   # 4  partition-tiles over inner
JT = N // P       # 8  key tiles
NT = N // P       # 8  output row tiles
NB = N // 512     # 2  free-dim blocks of 512 over N

F32 = mybir.dt.float32
F32R = mybir.dt.float32r
BF16 = mybir.dt.bfloat16
EXP = mybir.ActivationFunctionType.Exp

WARM_MM = 4       # dummy matmuls to touch the PE before real work
WARM_MM_GAP = 2   # dummies between kq0 accumulation steps (DMA-paced)

# (pair, slot) -> list of (src_pair, key_tile) attn@v steps to emit there.
AV_PLAN = {
    0: {3: [(0, 0)], 4: [(0, 1)], 5: [(0, 2)], 6: [(0, 3)],
        7: [(0, 4), (0, 5)]},
    1: {0: [(0, 6), (0, 7)], 2: [(1, 0)], 3: [(1, 1)], 4: [(1, 2)],
        5: [(1, 3)], 6: [(1, 4), (1, 5)], 7: [(1, 6)]},
    2: {0: [(1, 7)], 2: [(2, 0)], 3: [(2, 1)], 4: [(2, 2)],
        5: [(2, 3)], 6: [(2, 4), (2, 5)], 7: [(2, 6)]},
    3: {0: [(2, 7)], 2: [(3, 0)], 3: [(3, 1)], 4: [(3, 2)],
        5: [(3, 3)], 6: [(3, 4)], 7: [(3, 5)]},
}
# previous pair's denominator processing is staged across slots so the
# in-order DVE queue never blocks on DMA latency: PSUM copies + first two
# chain DMAs at slot 1, reciprocals + last two DMAs at slot 3, the
# normalize multiplies at slot 5.
DENOM_COPY_SLOT = 1
DENOM_RECIP_SLOT = 3
DENOM_MUL_SLOT = 5


def build_nc(debug=False):
    nc = bacc.Bacc(
        "TRN2", target_bir_lowering=False, debug=debug, num_devices=B
    )
    x_d = nc.dram_tensor("x", [C, N], F32, kind="ExternalInput")
    wq_d = nc.dram_tensor("Wq", [C, INNER], F32, kind="ExternalInput")
    wkv_d = nc.dram_tensor("Wkv", [C, 2 * INNER], F32, kind="ExternalInput")
    wo_d = nc.dram_tensor("Wo", [INNER, C], F32, kind="ExternalInput")
    bo_d = nc.dram_tensor("bo", [C], F32, kind="ExternalInput")
    out_d = nc.dram_tensor("out", [N, C], F32, kind="ExternalOutput")

    with tile.TileContext(nc) as tc:
        with (
            tc.tile_pool(name="persist", bufs=1) as persist,
            tc.tile_pool(name="stage", bufs=1) as stage,
            tc.tile_pool(name="etp", bufs=4) as etp,
            tc.tile_pool(name="small", bufs=2) as small,
            tc.tile_pool(name="dramp", bufs=2, space="DRAM") as dramp,
            tc.tile_pool(name="psS", bufs=2, space="PSUM") as psS,
            tc.tile_pool(name="psO", bufs=2, space="PSUM") as psO,
        ):
            # ---------------- constants + PE warmup ----------------
            zb = persist.tile([P, 1], F32)
            nc.vector.memset(zb, 0.0)
            zw = persist.tile([P, 512], BF16)
            nc.vector.memset(zw, 0.0)
            v_ext = persist.tile([P, JT, HEADS, D + 1], BF16)
            nc.vector.memset(v_ext[:, :, :, D], 1.0)

            warm = psO.tile([D + 1, N], F32, tag="ov")
            warm_i = [0]

            def warm_mm(k):
                for _ in range(k):
                    i = warm_i[0]
                    warm_i[0] += 1
                    nc.tensor.matmul(
                        warm[:, (i % 2) * 512:(i % 2) * 512 + 512],
                        lhsT=zw[:, 0:D + 1],
                        rhs=zw[:, 0:512],
                        start=True,
                        stop=True,
                    )

            warm_mm(WARM_MM)

            # ---------------- input DMA + casts (phase 1) --------------
            # kq0's gate: tiny strided column tiles of Wq / Wkv-k first,
            # then the x chunks (cast per chunk on DVE).
            wq0_f = stage.tile([P, CT, P], F32, tag="st_w0q")
            wq_b = persist.tile([P, MT, CT, P], BF16)
            wkvk0_f = stage.tile([P, CT, P], F32, tag="st_w0k")
            wkvk_b = persist.tile([P, MT, CT, P], BF16)
            nc.sync.dma_start(
                out=wq0_f, in_=wq_d[:, 0:P].rearrange("(a p) m -> p a m", p=P))
            nc.gpsimd.tensor_copy(
                out=wq_b[:, 0], in_=wq0_f)
            nc.sync.dma_start(
                out=wkvk0_f,
                in_=wkv_d[:, 0:P].rearrange("(a p) m -> p a m", p=P))
            nc.gpsimd.tensor_copy(
                out=wkvk_b[:, 0], in_=wkvk0_f)
            x_f = stage.tile([P, CT, N], F32, tag="st_x")
            x_b = persist.tile([P, CT, N], BF16)
            x_dv = x_d[:].rearrange("(a p) n -> p a n", p=P)
            for a in range(CT):
                nc.sync.dma_start(out=x_f[:, a, :], in_=x_dv[:, a, :])
                nc.vector.tensor_copy(out=x_b[:, a, :], in_=x_f[:, a, :])

            # persistent activation tiles: qT/kT (inner, N); head h lives at
            # partitions (h%2)*64, tile index h//2.
            qT = persist.tile([P, MT, N], BF16)
            kT = persist.tile([P, MT, N], BF16)
            oTs = []
            for m in range(MT):
                oTs.append(persist.tile([P, N], BF16, tag=f"oT{m}",
                                        name=f"oT{m}"))
            a_sb = persist.tile([P, NT, C], BF16)

            # ---------------- kq0: DMA-paced, warm-interleaved ----------
            st_k = psS.tile([P, N], F32, tag="st")
            st_q = psS.tile([P, N], F32, tag="st")
            for a in range(CT):
                for st, wb in ((st_q, wq_b), (st_k, wkvk_b)):
                    for ib in range(NB):
                        nc.tensor.matmul(
                            st[:, ib * 512:(ib + 1) * 512],
                            lhsT=wb[:, 0, a],
                            rhs=x_b[:, a, ib * 512:(ib + 1) * 512],
                            start=(a == 0),
                            stop=(a == CT - 1),
                        )
                warm_mm(WARM_MM_GAP)
            # readouts: qT on Scalar (idle until the first exp), kT on DVE
            # split in halves so the first sims unlock sooner.
            nc.scalar.copy(out=qT[:, 0, 0:512], in_=st_q[:, 0:512])
            nc.vector.tensor_copy(out=kT[:, 0, 0:512], in_=st_k[:, 0:512])
            nc.scalar.copy(out=qT[:, 0, 512:N], in_=st_q[:, 512:N])
            nc.vector.tensor_copy(out=kT[:, 0, 512:N], in_=st_k[:, 512:N])

            # ---------------- input DMA + casts (phase 2) --------------
            # wkv-v first (V groups run in the first attention slots); the
            # bulk Wq / Wkv-k / Wo DMAs are issued on the GpSimd queue
            # behind the wkv-v casts, so x + wkv-v get full DMA bandwidth
            # before the bulk weights start streaming.
            wkvv_b = persist.tile([P, CT, INNER], BF16)
            wkvv_f = stage.tile([P, CT, INNER], F32, tag="st_wv")
            nc.sync.dma_start(
                out=wkvv_f,
                in_=wkv_d[:, INNER:2 * INNER].rearrange("(a p) m -> p a m", p=P))
            # casts split across engines (concurrent Pool casts measured
            # ~1.9us each): Scalar is idle until the first exp, DVE takes
            # one, GpSimd keeps the last as the bulk-weight-DMA gate.
            # none on Scalar: anything here would sit between the qT
            # readouts and the first exp in the in-order Scalar queue,
            # delaying the whole exp stream by the wkv-v DMA wait
            nc.gpsimd.tensor_copy(out=wkvv_b[:, 0], in_=wkvv_f[:, 0])
            nc.vector.tensor_copy(out=wkvv_b[:, 1], in_=wkvv_f[:, 1])
            nc.vector.tensor_copy(out=wkvv_b[:, 2], in_=wkvv_f[:, 2])
            nc.gpsimd.tensor_copy(out=wkvv_b[:, 3], in_=wkvv_f[:, 3])
            wq_f = stage.tile([P, CT, INNER], F32, tag="st_w")
            nc.gpsimd.dma_start(
                out=wq_f, in_=wq_d[:].rearrange("(a p) m -> p a m", p=P))
            for mt in range(1, MT):
                nc.gpsimd.tensor_copy(
                    out=wq_b[:, mt], in_=wq_f[:, :, mt * P:(mt + 1) * P])
            wkvk_f = stage.tile([P, CT, INNER], F32, tag="st_w")
            nc.gpsimd.dma_start(
                out=wkvk_f,
                in_=wkv_d[:, 0:INNER].rearrange("(a p) m -> p a m", p=P))
            for mt in range(1, MT):
                nc.gpsimd.tensor_copy(
                    out=wkvk_b[:, mt], in_=wkvk_f[:, :, mt * P:(mt + 1) * P])
            wo_b = persist.tile([P, MT, C], BF16)
            wo_f = stage.tile([P, CT, C], F32, tag="st_wo")
            nc.gpsimd.dma_start(
                out=wo_f,
                in_=wo_d[:].rearrange("(a p) m -> p a m", p=P))
            nc.gpsimd.tensor_copy(out=wo_b, in_=wo_f)

            bo_bc = persist.tile([P, C], F32)
            bo_ap = bo_d[:]
            nc.gpsimd.dma_start(
                out=bo_bc,
                in_=bass.AP(tensor=bo_ap.tensor, offset=bo_ap.offset,
                            ap=[[0, P], [1, C]]),
            )

            # ---------------- slot work groups (~4 matmuls each) --------
            def kq_group(dst, wb, mt, ib):
                ps = psS.tile([P, N], F32, tag="st", name=f"kq{mt}_{ib}")
                for a in range(CT):
                    nc.tensor.matmul(
                        ps[:, ib * 512:(ib + 1) * 512],
                        lhsT=wb[:, mt, a],
                        rhs=x_b[:, a, ib * 512:(ib + 1) * 512],
                        start=(a == 0),
                        stop=(a == CT - 1),
                    )
                nc.vector.tensor_copy(
                    out=dst[:, mt, ib * 512:(ib + 1) * 512],
                    in_=ps[:, ib * 512:(ib + 1) * 512])

            def v_group(jts):
                for jt in jts:
                    ps = psS.tile([P, N], F32, tag="st", name=f"v{jt}")
                    for a in range(CT):
                        nc.tensor.matmul(
                            ps[:, 0:512],
                            lhsT=x_b[:, a, jt * P:(jt + 1) * P],
                            rhs=wkvv_b[:, a, :],
                            start=(a == 0),
                            stop=(a == CT - 1),
                        )
                    nc.vector.tensor_copy(
                        out=v_ext[:, jt, :, 0:D],
                        in_=ps[:, 0:512].rearrange("p (h d) -> p h d", h=HEADS),
                    )

            def a_group(it):
                # output projection partial m=0..2 for row tile it,
                # accumulated (+bias) into SBUF; the tail adds only m=3.
                ps = psS.tile([P, N], F32, tag="st", name=f"a{it}")
                for kk in range(MT - 1):
                    nc.tensor.matmul(
                        ps[:, 0:C],
                        lhsT=oTs[kk][:, it * P:(it + 1) * P],
                        rhs=wo_b[:, kk, :],
                        start=(kk == 0),
                        stop=(kk == MT - 2),
                    )
                nc.vector.tensor_add(a_sb[:, it, :], ps[:, 0:C], bo_bc)

            # (pair, slot) -> work groups; PREFIX groups run before the
            # slot's sims (the pair-1 sims need kq1 complete).
            CHUNKS = {
                (0, 1): lambda: v_group((0, 1)),
                (0, 2): lambda: v_group((2, 3)),
                (0, 3): lambda: v_group((4, 5)),
                (0, 4): lambda: v_group((6, 7)),
                (0, 5): lambda: (kq_group(qT, wq_b, 1, 0),
                                 kq_group(qT, wq_b, 1, 1)),
                (0, 7): lambda: (kq_group(kT, wkvk_b, 1, 0),
                                 kq_group(kT, wkvk_b, 1, 1)),
                (1, 1): lambda: (kq_group(kT, wkvk_b, 2, 0),
                                 kq_group(qT, wq_b, 2, 0)),
                (1, 3): lambda: (kq_group(kT, wkvk_b, 2, 1),
                                 kq_group(qT, wq_b, 2, 1)),
                (2, 0): lambda: (kq_group(kT, wkvk_b, 3, 0),
                                 kq_group(qT, wq_b, 3, 0)),
                (2, 2): lambda: (kq_group(kT, wkvk_b, 3, 1),
                                 kq_group(qT, wq_b, 3, 1)),
                (3, 6): lambda: (a_group(0), a_group(1)),
            }
            PREFIX_CHUNKS = {}

            # ---------------- attention: head pairs ----------------
            ovs = {}
            ets = {}

            def sim_head(hm, jt, i):
                hp = i * D
                st = psS.tile([P, N], F32, tag="st", name=f"sim{hm}_{jt}_{i}")
                for ib in range(NB):
                    nc.tensor.matmul(
                        st[:, ib * 512:(ib + 1) * 512],
                        lhsT=kT[hp:hp + D, hm, jt * P:(jt + 1) * P],
                        rhs=qT[hp:hp + D, hm, ib * 512:(ib + 1) * 512],
                        start=True,
                        stop=True,
                    )
                nc.scalar.activation(
                    out=ets[2 * hm + i][:, jt, :], in_=st, func=EXP,
                    bias=zb, scale=SCALE)

            def sim_pair(hm, jt):
                sts = [psS.tile([P, N], F32, tag="st", name=f"sim{hm}_{jt}_{i}")
                       for i in range(2)]
                for ib in range(NB):
                    for i, hp in enumerate((0, 64)):
                        nc.tensor.matmul(
                            sts[i][:, ib * 512:(ib + 1) * 512],
                            lhsT=kT[hp:hp + D, hm, jt * P:(jt + 1) * P],
                            rhs=qT[hp:hp + D, hm, ib * 512:(ib + 1) * 512],
                            start=True,
                            stop=True,
                        )
                for i in range(2):
                    h = 2 * hm + i
                    nc.scalar.activation(
                        out=ets[h][:, jt, :], in_=sts[i], func=EXP,
                        bias=zb, scale=SCALE)

            def av_head(h, jt):
                ov = ovs[h]
                et = ets[h]
                for ib in range(NB):
                    nc.tensor.matmul(
                        ov[:, ib * 512:(ib + 1) * 512],
                        lhsT=v_ext[:, jt, h, :],
                        rhs=et[:, jt, ib * 512:(ib + 1) * 512],
                        start=(jt == 0),
                        stop=(jt == JT - 1),
                    )

            def av_pair(hm, jt):
                av_head(2 * hm, jt)
                av_head(2 * hm + 1, jt)

            def recip_chain(src_row, rep, dma_eng):
                # [1,N] denom row (SBUF) -> DRAM -> [128,8] -> recip ->
                # DRAM -> [D,N] broadcast
                sd = dramp.tile([N], F32, tag="sd")
                dma_eng.dma_start(out=sd, in_=src_row)
                s_sp = small.tile([P, NT], F32, tag="s_sp")
                dma_eng.dma_start(
                    out=s_sp, in_=sd.rearrange("(p k) -> p k", k=NT))
                r_sp = small.tile([P, NT], F32, tag="r_sp")
                nc.vector.reciprocal(r_sp, s_sp)
                rd = dramp.tile([N], F32, tag="rd")
                dma_eng.dma_start(
                    out=rd.rearrange("(p k) -> p k", k=NT), in_=r_sp)
                rd_ap = rd[:]
                dma_eng.dma_start(
                    out=rep,
                    in_=bass.AP(tensor=rd_ap.tensor, offset=rd_ap.offset,
                                ap=[[0, D], [1, N]]),
                )

            # staged denominator processing (see DENOM_*_SLOT comments)
            dstate = {}

            def denom_copy(h):
                # copy O'+denom out of PSUM (frees the ov bank) and start
                # the DRAM bounce of the denom row
                ov = ovs.pop(h)
                ov_sb = small.tile([D + 1, N], F32, tag="ovsb",
                                   name=f"ovsb{h}")
                nc.vector.tensor_copy(out=ov_sb, in_=ov)
                sd = dramp.tile([N], F32, tag="sd", name=f"sd{h}")
                nc.sync.dma_start(out=sd, in_=ov_sb[D:D + 1, :])
                s_sp = small.tile([P, NT], F32, tag="s_sp", name=f"ssp{h}")
                nc.sync.dma_start(
                    out=s_sp, in_=sd.rearrange("(p k) -> p k", k=NT))
                dstate[h] = (ov_sb, s_sp)

            def denom_recip(h):
                ov_sb, s_sp = dstate[h]
                r_sp = small.tile([P, NT], F32, tag="r_sp", name=f"rsp{h}")
                nc.vector.reciprocal(r_sp, s_sp)
                rd = dramp.tile([N], F32, tag="rd", name=f"rd{h}")
                nc.sync.dma_start(
                    out=rd.rearrange("(p k) -> p k", k=NT), in_=r_sp)
                rep = small.tile([D, N], F32, tag="rep", name=f"rep{h}")
                rd_ap = rd[:]
                nc.sync.dma_start(
                    out=rep,
                    in_=bass.AP(tensor=rd_ap.tensor, offset=rd_ap.offset,
                                ap=[[0, D], [1, N]]),
                )
                dstate[h] = (ov_sb, rep)

            def denom_mul(h, mul_eng=None):
                ov_sb, rep = dstate.pop(h)
                hp = (h % 2) * D
                eng = mul_eng or nc.vector
                eng.tensor_mul(oTs[h // 2][hp:hp + D, :], ov_sb[0:D, :], rep)
                del ets[h]

            for hm in range(MT):
                h0, h1 = 2 * hm, 2 * hm + 1
                ets[h0] = etp.tile([P, JT, N], BF16, tag="et", name=f"et{h0}")
                ets[h1] = etp.tile([P, JT, N], BF16, tag="et", name=f"et{h1}")
                plan = AV_PLAN[hm]
                for jt in range(JT):
                    sim_pair(hm, jt)
                    if hm > 0:
                        if jt == DENOM_COPY_SLOT:
                            denom_copy(h0 - 2)
                            denom_copy(h1 - 2)
                        elif jt == DENOM_RECIP_SLOT:
                            denom_recip(h0 - 2)
                            denom_recip(h1 - 2)
                        elif jt == DENOM_MUL_SLOT:
                            denom_mul(h0 - 2)
                            denom_mul(h1 - 2, mul_eng=nc.gpsimd)
                    if jt == 0:
                        ovs[h0] = psO.tile([D + 1, N], F32, tag="ov",
                                           name=f"ov{h0}")
                        ovs[h1] = psO.tile([D + 1, N], F32, tag="ov",
                                           name=f"ov{h1}")
                    for sp, kt in plan.get(jt, ()):
                        av_pair(sp, kt)
                    ch = CHUNKS.get((hm, jt))
                    if ch is not None:
                        ch()

            # ---------------- tail ----------------
            # Late normalization for the last pair: the reciprocal comes
            # back COLUMN-major ([128, 8]: partition q%128, column q//128)
            # after only 2 DMA hops, and is applied as a per-partition
            # Scalar scale on per-head output-projection partials -- no
            # 64-row broadcast needed. a-groups keep the PE warm.
            h6, h7 = HEADS - 2, HEADS - 1
            av_pair(3, JT - 2)
            av_head(h6, JT - 1)
            row6_t = small.tile([D, N], F32, tag="rep", name="row6")
            row6 = row6_t[0:1, :]
            nc.scalar.copy(out=row6, in_=ovs[h6][D:D + 1, :])
            sd6 = dramp.tile([N], F32, tag="sd", name="sd6")
            nc.sync.dma_start(out=sd6, in_=row6)
            s6c = small.tile([P, NT], F32, tag="s_sp", name="s6c")
            nc.sync.dma_start(
                out=s6c, in_=sd6.rearrange("(k p) -> p k", p=P))
            av_head(h7, JT - 1)
            row7_t = small.tile([D, N], F32, tag="rep", name="row7")
            row7 = row7_t[0:1, :]
            nc.scalar.copy(out=row7, in_=ovs[h7][D:D + 1, :])
            sd7 = dramp.tile([N], F32, tag="sd", name="sd7")
            nc.sync.dma_start(out=sd7, in_=row7)
            s7c = small.tile([P, NT], F32, tag="s_sp", name="s7c")
            nc.sync.dma_start(
                out=s7c, in_=sd7.rearrange("(k p) -> p k", p=P))

            for it in range(2, NT):
                a_group(it)

            # unnormalized O' for both heads (lhsT of the B matmuls)
            oT3u = persist.tile([P, N], BF16)
            ov6 = ovs.pop(h6)
            nc.vector.tensor_copy(out=oT3u[0:D, :], in_=ov6[0:D, :])
            ov7 = ovs.pop(h7)
            nc.vector.tensor_copy(out=oT3u[D:P, :], in_=ov7[0:D, :])
            r6c = small.tile([P, NT], F32, tag="r_sp", name="r6c")
            nc.vector.reciprocal(r6c, s6c)
            r7c = small.tile([P, NT], F32, tag="r_sp", name="r7c")
            nc.vector.reciprocal(r7c, s7c)
            del ets[h6], ets[h7]

            # per output tile: two K=64 head-partials (alternating PE row
            # tiles), Scalar applies 1/denom per partition, DVE folds in
            # the m=0..2+bias partial
            for it in range(NT):
                pb = psS.tile([P, N], F32, tag="st", name=f"pb{it}")
                for half, hp in ((0, 0), (1, D)):
                    nc.tensor.matmul(
                        pb[:, half * 512:half * 512 + C],
                        lhsT=oT3u[hp:hp + D, it * P:(it + 1) * P],
                        rhs=wo_b[hp:hp + D, 3, :],
                        start=True,
                        stop=True,
                    )
                t6 = small.tile([P, C], F32, tag="t6", name=f"t6_{it}")
                nc.scalar.activation(
                    out=t6, in_=pb[:, 0:C],
                    func=mybir.ActivationFunctionType.Copy,
                    scale=r6c[:, it:it + 1])
                t67 = small.tile([P, C], F32, tag="t7", name=f"t67_{it}")
                nc.vector.scalar_tensor_tensor(
                    out=t67, in0=pb[:, 512:512 + C],
                    scalar=r7c[:, it:it + 1], in1=t6,
                    op0=mybir.AluOpType.mult, op1=mybir.AluOpType.add)
                fin = small.tile([P, C], F32, tag="fin", bufs=3)
                eng = nc.vector if it % 2 == 0 else nc.gpsimd
                eng.tensor_add(fin, t67, a_sb[:, it, :])
                nc.sync.dma_start(out=out_d[it * P:(it + 1) * P, :], in_=fin)

    return nc


def kernel(x, Wq, Wkv, Wo, bo):
    from concourse.bass_utils import run_bass_kernel_spmd

    nc = build_nc()
    nc.compile()
    x = np.asarray(x)
    xs = np.ascontiguousarray(x.reshape(B, C, N)).astype(np.float32, copy=False)
    in_maps = [
        {
            "x": xs[b],
            "Wq": np.asarray(Wq, dtype=np.float32),
            "Wkv": np.asarray(Wkv, dtype=np.float32),
            "Wo": np.asarray(Wo, dtype=np.float32),
            "bo": np.asarray(bo, dtype=np.float32),
        }
        for b in range(B)
    ]
    res = run_bass_kernel_spmd(nc, in_maps, list(range(B)))
    return np.stack([res.results[b]["out"] for b in range(B)], axis=0)


# revision 5
# speedup vs baseline: 1.1996x; 1.0067x over previous
"""Self-contained Trainium2 Bass kernel for nn_Attention (8-head self-attention).

Reference computation (per batch element b):
    xt = x[b].reshape(C, N).T            # (N, C),  N = H*W = 1024
    q  = xt @ Wq                         # (N, 512)
    k, v = split(xt @ Wkv)               # (N, 512) each
    per head h (d=64): sim = q_h k_h^T / 8 ; P = softmax(sim) ; o_h = P v_h
    out[b] = concat_h(o_h) @ Wo + bo     # (N, C)

Sharding: pure data parallel -- core b computes batch element b (8 cores, 8
batch elements, no collectives).

Performance architecture (measured on HW):
  - Scalar exp throughput is the pacing floor: 64 exps of [128,1024]
    ~= 71.5us. The schedule keeps the exp stream as gapless as possible;
    Scalar runs exps ONLY until the tail (casts on DVE / GpSimd).
  - PE matmuls on DISJOINT row tiles execute concurrently (measured
    212ns/mm for alternating [64,128] tiles vs 588ns same-tile). Even/odd
    heads live on partitions 0:64 / 64:128 of qT/kT; sim matmuls are
    emitted head-alternating so the K=64 sim phase runs at ~full PE rate.
  - Head PAIRS are processed in 8 key-tile slots (2 exps each). attn@v,
    the remaining projections, and the early output-projection partials
    are placed into specific slots (AV_PLAN / CHUNKS) in ~4-matmul groups
    so per-slot PE work tracks the ~2.2us Scalar slot and every group's
    inputs (DMA arrivals, exp completions, PSUM frees) land just ahead.
  - Inputs load as big contiguous DMAs (x chunks, then full Wq / Wkv-k /
    Wkv-v / Wo row blocks, cast-sliced on GpSimd) plus two tiny strided
    column-tile DMAs so pair 0 gates only on x + 1/4 of Wq + 1/4 of Wkv.
  - Softmax denominators ride row 64 of the attn@v PSUM accumulator
    (ones column in v_ext). Reciprocals must run spread across 128
    partitions (serial ~6.4ns/elem per lane otherwise): DRAM bounce to
    [128,8], recip, bounce back for the 64-row broadcast. The two tail
    chains run on separate DMA queues with Scalar doing the PSUM row
    copies. Output projection for m=0..2 (+bias) is pre-accumulated into
    SBUF during pair 3, so after the final normalize each output tile is
    one matmul + one DVE add away from its DMA.
"""

import numpy as np

import concourse.bass as bass
import concourse.mybir as mybir
import concourse.tile as tile
from concourse import bacc

B, C, N = 8, 512, 1024
HEADS, D = 8, 64
INNER = HEADS * D  # 512
SCALE = D ** -0.5
P = 128
CT = C // P       # 4  k-tiles over C
MT = INNER // P   # 4  partition-tiles over inner
JT = N // P       # 8  key tiles
NT = N // P       # 8  output row tiles
NB = N // 512     # 2  free-dim blocks of 512 over N

F32 = mybir.dt.float32
F32R = mybir.dt.float32r
BF16 = mybir.dt.bfloat16
EXP = mybir.ActivationFunctionType.Exp

WARM_MM = 4       # dummy matmuls to touch the PE before real work
WARM_MM_GAP = 2   # dummies between kq0 accumulation steps (DMA-paced)

# (pair, slot) -> list of (src_pair, key_tile) attn@v steps to emit there.
AV_PLAN = {
    0: {3: [(0, 0)], 4: [(0, 1)], 5: [(0, 2)], 6: [(0, 3)],
        7: [(0, 4), (0, 5)]},
    1: {0: [(0, 6), (0, 7)], 2: [(1, 0)], 3: [(1, 1)], 4: [(1, 2)],
        5: [(1, 3)], 6: [(1, 4), (1, 5)], 7: [(1, 6)]},
    2: {0: [(1, 7)], 2: [(2, 0)], 3: [(2, 1)], 4: [(2, 2)],
        5: [(2, 3)], 6: [(2, 4), (2, 5)], 7: [(2, 6)]},
    3: {0: [(2, 7)], 2: [(3, 0)], 3: [(3, 1)], 4: [(3, 2)],
        5: [(3, 3)], 6: [(3, 4)], 7: [(3, 5)]},
}
# previous pair's denominator processing is staged across slots so the
# in-order DVE queue never blocks on DMA latency: PSUM copies + first two
# chain DMAs at slot 1, reciprocals + last two DMAs at slot 3, the
# normalize multiplies at slot 5.
DENOM_COPY_SLOT = 1
DENOM_RECIP_SLOT = 3
DENOM_MUL_SLOT = 5


def build_nc(debug=False):
    nc = bacc.Bacc(
        "TRN2", target_bir_lowering=False, debug=debug, num_devices=B
    )
    x_d = nc.dram_tensor("x", [C, N], F32, kind="ExternalInput")
    wq_d = nc.dram_tensor("Wq", [C, INNER], F32, kind="ExternalInput")
    wkv_d = nc.dram_tensor("Wkv", [C, 2 * INNER], F32, kind="ExternalInput")
    wo_d = nc.dram_tensor("Wo", [INNER, C], F32, kind="ExternalInput")
    bo_d = nc.dram_tensor("bo", [C], F32, kind="ExternalInput")
    out_d = nc.dram_tensor("out", [N, C], F32, kind="ExternalOutput")

    with tile.TileContext(nc) as tc:
        with (
            tc.tile_pool(name="persist", bufs=1) as persist,
            tc.tile_pool(name="stage", bufs=1) as stage,
            tc.tile_pool(name="etp", bufs=4) as etp,
            tc.tile_pool(name="small", bufs=2) as small,
            tc.tile_pool(name="dramp", bufs=2, space="DRAM") as dramp,
            tc.tile_pool(name="psS", bufs=2, space="PSUM") as psS,
            tc.tile_pool(name="psO", bufs=2, space="PSUM") as psO,
        ):
            # ---------------- constants + PE warmup ----------------
            zb = persist.tile([P, 1], F32)
            nc.vector.memset(zb, 0.0)
            zw = persist.tile([P, 512], BF16)
            nc.vector.memset(zw, 0.0)
            v_ext = persist.tile([P, JT, HEADS, D + 1], BF16)
            nc.vector.memset(v_ext[:, :, :, D], 1.0)

            warm = psO.tile([D + 1, N], F32, tag="ov")
            warm_i = [0]

            def warm_mm(k):
                for _ in range(k):
                    i = warm_i[0]
                    warm_i[0] += 1
                    nc.tensor.matmul(
                        warm[:, (i % 2) * 512:(i % 2) * 512 + 512],
                        lhsT=zw[:, 0:D + 1],
                        rhs=zw[:, 0:512],
                        start=True,
                        stop=True,
                    )

            warm_mm(WARM_MM)

            # ---------------- input DMA + casts (phase 1) --------------
            # kq0's gate: tiny strided column tiles of Wq / Wkv-k first,
            # then the x chunks (cast per chunk on DVE).
            wq0_f = stage.tile([P, CT, P], F32, tag="st_w0q")
            wq_b = persist.tile([P, MT, CT, P], BF16)
            wkvk0_f = stage.tile([P, CT, P], F32, tag="st_w0k")
            wkvk_b = persist.tile([P, MT, CT, P], BF16)
            nc.sync.dma_start(
                out=wq0_f, in_=wq_d[:, 0:P].rearrange("(a p) m -> p a m", p=P))
            nc.gpsimd.tensor_copy(
                out=wq_b[:, 0], in_=wq0_f)
            nc.sync.dma_start(
                out=wkvk0_f,
                in_=wkv_d[:, 0:P].rearrange("(a p) m -> p a m", p=P))
            nc.gpsimd.tensor_copy(
                out=wkvk_b[:, 0], in_=wkvk0_f)
            x_f = stage.tile([P, CT, N], F32, tag="st_x")
            x_b = persist.tile([P, CT, N], BF16)
            x_dv = x_d[:].rearrange("(a p) n -> p a n", p=P)
            for a in range(CT):
                nc.sync.dma_start(out=x_f[:, a, :], in_=x_dv[:, a, :])
                nc.vector.tensor_copy(out=x_b[:, a, :], in_=x_f[:, a, :])

            # persistent activation tiles: qT/kT (inner, N); head h lives at
            # partitions (h%2)*64, tile index h//2.
            qT = persist.tile([P, MT, N], BF16)
            kT = persist.tile([P, MT, N], BF16)
            oTs = []
            for m in range(MT):
                oTs.append(persist.tile([P, N], BF16, tag=f"oT{m}",
                                        name=f"oT{m}"))
            a_sb = persist.tile([P, NT, C], BF16)

            # ---------------- kq0: DMA-paced, warm-interleaved ----------
            st_k = psS.tile([P, N], F32, tag="st")
            st_q = psS.tile([P, N], F32, tag="st")
            for a in range(CT):
                for st, wb in ((st_q, wq_b), (st_k, wkvk_b)):
                    for ib in range(NB):
                        nc.tensor.matmul(
                            st[:, ib * 512:(ib + 1) * 512],
                            lhsT=wb[:, 0, a],
                            rhs=x_b[:, a, ib * 512:(ib + 1) * 512],
                            start=(a == 0),
                            stop=(a == CT - 1),
                        )
                warm_mm(WARM_MM_GAP)
            # readouts: qT on Scalar (idle until the first exp), kT on DVE
            # split in halves so the first sims unlock sooner.
            nc.scalar.copy(out=qT[:, 0, 0:512], in_=st_q[:, 0:512])
            nc.vector.tensor_copy(out=kT[:, 0, 0:512], in_=st_k[:, 0:512])
            nc.scalar.copy(out=qT[:, 0, 512:N], in_=st_q[:, 512:N])
            nc.vector.tensor_copy(out=kT[:, 0, 512:N], in_=st_k[:, 512:N])

            # ---------------- input DMA + casts (phase 2) --------------
            # wkv-v first (V groups run in the first attention slots); the
            # bulk Wq / Wkv-k / Wo DMAs are issued on the GpSimd queue
            # behind the wkv-v casts, so x + wkv-v get full DMA bandwidth
            # before the bulk weights start streaming.
            wkvv_b = persist.tile([P, CT, INNER], BF16)
            wkvv_f = stage.tile([P, CT, INNER], F32, tag="st_wv")
            nc.sync.dma_start(
                out=wkvv_f,
                in_=wkv_d[:, INNER:2 * INNER].rearrange("(a p) m -> p a m", p=P))
            # casts split across engines (concurrent Pool casts measured
            # ~1.9us each): Scalar is idle until the first exp, DVE takes
            # one, GpSimd keeps the last as the bulk-weight-DMA gate.
            # none on Scalar: anything here would sit between the qT
            # readouts and the first exp in the in-order Scalar queue,
            # delaying the whole exp stream by the wkv-v DMA wait
            nc.gpsimd.tensor_copy(out=wkvv_b[:, 0], in_=wkvv_f[:, 0])
            nc.vector.tensor_copy(out=wkvv_b[:, 1], in_=wkvv_f[:, 1])
            nc.vector.tensor_copy(out=wkvv_b[:, 2], in_=wkvv_f[:, 2])
            nc.gpsimd.tensor_copy(out=wkvv_b[:, 3], in_=wkvv_f[:, 3])
            wq_f = stage.tile([P, CT, INNER], F32, tag="st_w")
            nc.gpsimd.dma_start(
                out=wq_f, in_=wq_d[:].rearrange("(a p) m -> p a m", p=P))
            for mt in range(1, MT):
                nc.gpsimd.tensor_copy(
                    out=wq_b[:, mt], in_=wq_f[:, :, mt * P:(mt + 1) * P])
            wkvk_f = stage.tile([P, CT, INNER], F32, tag="st_w")
            nc.gpsimd.dma_start(
                out=wkvk_f,
                in_=wkv_d[:, 0:INNER].rearrange("(a p) m -> p a m", p=P))
            for mt in range(1, MT):
                nc.gpsimd.tensor_copy(
                    out=wkvk_b[:, mt], in_=wkvk_f[:, :, mt * P:(mt + 1) * P])
            wo_b = persist.tile([P, MT, C], BF16)
            wo_f = stage.tile([P, CT, C], F32, tag="st_wo")
            nc.gpsimd.dma_start(
                out=wo_f,
                in_=wo_d[:].rearrange("(a p) m -> p a m", p=P))
            nc.gpsimd.tensor_copy(out=wo_b, in_=wo_f)

            bo_bc = persist.tile([P, C], F32)
            bo_ap = bo_d[:]
            nc.gpsimd.dma_start(
                out=bo_bc,
                in_=bass.AP(tensor=bo_ap.tensor, offset=bo_ap.offset,
                            ap=[[0, P], [1, C]]),
            )

            # ---------------- slot work groups (~4 matmuls each) --------
            def kq_group(dst, wb, mt, ib):
                ps = psS.tile([P, N], F32, tag="st", name=f"kq{mt}_{ib}")
                for a in range(CT):
                    nc.tensor.matmul(
                        ps[:, ib * 512:(ib + 1) * 512],
                        lhsT=wb[:, mt, a],
                        rhs=x_b[:, a, ib * 512:(ib + 1) * 512],
                        start=(a == 0),
                        stop=(a == CT - 1),
                    )
                nc.vector.tensor_copy(
                    out=dst[:, mt, ib * 512:(ib + 1) * 512],
                    in_=ps[:, ib * 512:(ib + 1) * 512])

            def v_group(jts):
                for jt in jts:
                    ps = psS.tile([P, N], F32, tag="st", name=f"v{jt}")
                    for a in range(CT):
                        nc.tensor.matmul(
                            ps[:, 0:512],
                            lhsT=x_b[:, a, jt * P:(jt + 1) * P],
                            rhs=wkvv_b[:, a, :],
                            start=(a == 0),
                            stop=(a == CT - 1),
                        )
                    nc.vector.tensor_copy(
                        out=v_ext[:, jt, :, 0:D],
                        in_=ps[:, 0:512].rearrange("p (h d) -> p h d", h=HEADS),
                    )

            def a_group(it):
                # output projection partial m=0..2 for row tile it,
                # accumulated (+bias) into SBUF; the tail adds only m=3.
                ps = psS.tile([P, N], F32, tag="st", name=f"a{it}")
                for kk in range(MT - 1):
                    nc.tensor.matmul(
                        ps[:, 0:C],
                        lhsT=oTs[kk][:, it * P:(it + 1) * P],
                        rhs=wo_b[:, kk, :],
                        start=(kk == 0),
                        stop=(kk == MT - 2),
                    )
                nc.vector.tensor_add(a_sb[:, it, :], ps[:, 0:C], bo_bc)

            # (pair, slot) -> work groups; PREFIX groups run before the
            # slot's sims (the pair-1 sims need kq1 complete).
            CHUNKS = {
                (0, 1): lambda: v_group((0, 1)),
                (0, 2): lambda: v_group((2, 3)),
                (0, 3): lambda: v_group((4, 5)),
                (0, 4): lambda: v_group((6, 7)),
                (0, 5): lambda: (kq_group(qT, wq_b, 1, 0),
                                 kq_group(qT, wq_b, 1, 1)),
                (0, 7): lambda: (kq_group(kT, wkvk_b, 1, 0),
                                 kq_group(kT, wkvk_b, 1, 1)),
                (1, 1): lambda: (kq_group(kT, wkvk_b, 2, 0),
                                 kq_group(qT, wq_b, 2, 0)),
                (1, 3): lambda: (kq_group(kT, wkvk_b, 2, 1),
                                 kq_group(qT, wq_b, 2, 1)),
                (2, 0): lambda: (kq_group(kT, wkvk_b, 3, 0),
                                 kq_group(qT, wq_b, 3, 0)),
                (2, 2): lambda: (kq_group(kT, wkvk_b, 3, 1),
                                 kq_group(qT, wq_b, 3, 1)),
                (3, 6): lambda: (a_group(0), a_group(1)),
            }
            PREFIX_CHUNKS = {}

            # ---------------- attention: head pairs ----------------
            ovs = {}
            ets = {}

            def sim_head(hm, jt, i):
                hp = i * D
                st = psS.tile([P, N], F32, tag="st", name=f"sim{hm}_{jt}_{i}")
                for ib in range(NB):
                    nc.tensor.matmul(
                        st[:, ib * 512:(ib + 1) * 512],
                        lhsT=kT[hp:hp + D, hm, jt * P:(jt + 1) * P],
                        rhs=qT[hp:hp + D, hm, ib * 512:(ib + 1) * 512],
                        start=True,
                        stop=True,
                    )
                nc.scalar.activation(
                    out=ets[2 * hm + i][:, jt, :], in_=st, func=EXP,
                    bias=zb, scale=SCALE)

            def sim_pair(hm, jt):
                sts = [psS.tile([P, N], F32, tag="st", name=f"sim{hm}_{jt}_{i}")
                       for i in range(2)]
                for ib in range(NB):
                    for i, hp in enumerate((0, 64)):
                        nc.tensor.matmul(
                            sts[i][:, ib * 512:(ib + 1) * 512],
                            lhsT=kT[hp:hp + D, hm, jt * P:(jt + 1) * P],
                            rhs=qT[hp:hp + D, hm, ib * 512:(ib + 1) * 512],
                            start=True,
                            stop=True,
                        )
                for i in range(2):
                    h = 2 * hm + i
                    nc.scalar.activation(
                        out=ets[h][:, jt, :], in_=sts[i], func=EXP,
                        bias=zb, scale=SCALE)

            def av_head(h, jt):
                ov = ovs[h]
                et = ets[h]
                for ib in range(NB):
                    nc.tensor.matmul(
                        ov[:, ib * 512:(ib + 1) * 512],
                        lhsT=v_ext[:, jt, h, :],
                        rhs=et[:, jt, ib * 512:(ib + 1) * 512],
                        start=(jt == 0),
                        stop=(jt == JT - 1),
                    )

            def av_pair(hm, jt):
                av_head(2 * hm, jt)
                av_head(2 * hm + 1, jt)

            def recip_chain(src_row, rep, dma_eng):
                # [1,N] denom row (SBUF) -> DRAM -> [128,8] -> recip ->
                # DRAM -> [D,N] broadcast
                sd = dramp.tile([N], F32, tag="sd")
                dma_eng.dma_start(out=sd, in_=src_row)
                s_sp = small.tile([P, NT], F32, tag="s_sp")
                dma_eng.dma_start(
                    out=s_sp, in_=sd.rearrange("(p k) -> p k", k=NT))
                r_sp = small.tile([P, NT], F32, tag="r_sp")
                nc.vector.reciprocal(r_sp, s_sp)
                rd = dramp.tile([N], F32, tag="rd")
                dma_eng.dma_start(
                    out=rd.rearrange("(p k) -> p k", k=NT), in_=r_sp)
                rd_ap = rd[:]
                dma_eng.dma_start(
                    out=rep,
                    in_=bass.AP(tensor=rd_ap.tensor, offset=rd_ap.offset,
                                ap=[[0, D], [1, N]]),
                )

            # staged denominator processing (see DENOM_*_SLOT comments)
            dstate = {}

            def denom_copy(h):
                # copy O'+denom out of PSUM (frees the ov bank) and start
                # the DRAM bounce of the denom row
                ov = ovs.pop(h)
                ov_sb = small.tile([D + 1, N], F32, tag="ovsb",
                                   name=f"ovsb{h}")
                nc.vector.tensor_copy(out=ov_sb, in_=ov)
                sd = dramp.tile([N], F32, tag="sd", name=f"sd{h}")
                nc.sync.dma_start(out=sd, in_=ov_sb[D:D + 1, :])
                s_sp = small.tile([P, NT], F32, tag="s_sp", name=f"ssp{h}")
                nc.sync.dma_start(
                    out=s_sp, in_=sd.rearrange("(p k) -> p k", k=NT))
                dstate[h] = (ov_sb, s_sp)

            def denom_recip(h):
                ov_sb, s_sp = dstate[h]
                r_sp = small.tile([P, NT], F32, tag="r_sp", name=f"rsp{h}")
                nc.vector.reciprocal(r_sp, s_sp)
                rd = dramp.tile([N], F32, tag="rd", name=f"rd{h}")
                nc.sync.dma_start(
                    out=rd.rearrange("(p k) -> p k", k=NT), in_=r_sp)
                rep = small.tile([D, N], F32, tag="rep", name=f"rep{h}")
                rd_ap = rd[:]
                nc.sync.dma_start(
                    out=rep,
                    in_=bass.AP(tensor=rd_ap.tensor, offset=rd_ap.offset,
                                ap=[[0, D], [1, N]]),
                )
                dstate[h] = (ov_sb, rep)

            def denom_mul(h, mul_eng=None):
                ov_sb, rep = dstate.pop(h)
                hp = (h % 2) * D
                eng = mul_eng or nc.vector
                eng.tensor_mul(oTs[h // 2][hp:hp + D, :], ov_sb[0:D, :], rep)
                del ets[h]

            for hm in range(MT):
                h0, h1 = 2 * hm, 2 * hm + 1
                ets[h0] = etp.tile([P, JT, N], BF16, tag="et", name=f"et{h0}")
                ets[h1] = etp.tile([P, JT, N], BF16, tag="et", name=f"et{h1}")
                plan = AV_PLAN[hm]
                for jt in range(JT):
                    sim_pair(hm, jt)
                    if hm > 0:
                        if jt == DENOM_COPY_SLOT:
                            denom_copy(h0 - 2)
                            denom_copy(h1 - 2)
                        elif jt == DENOM_RECIP_SLOT:
                            denom_recip(h0 - 2)
                            denom_recip(h1 - 2)
                        elif jt == DENOM_MUL_SLOT:
                            denom_mul(h0 - 2)
                            denom_mul(h1 - 2, mul_eng=nc.gpsimd)
                    if jt == 0:
                        ovs[h0] = psO.tile([D + 1, N], F32, tag="ov",
                                           name=f"ov{h0}")
                        ovs[h1] = psO.tile([D + 1, N], F32, tag="ov",
                                           name=f"ov{h1}")
                    for sp, kt in plan.get(jt, ()):
                        av_pair(sp, kt)
                    ch = CHUNKS.get((hm, jt))
                    if ch is not None:
                        ch()

            # ---------------- tail ----------------
            # Late normalization for the last pair: the reciprocal comes
            # back COLUMN-major ([128, 8]: partition q%128, column q//128)
            # after only 2 DMA hops, and is applied as a per-partition
            # Scalar scale on per-head output-projection partials -- no
            # 64-row broadcast needed. a-groups keep the PE warm.
            h6, h7 = HEADS - 2, HEADS - 1
            av_head(h6, JT - 2)
            av_head(h6, JT - 1)
            row6_t = small.tile([D, N], F32, tag="rep", name="row6")
            row6 = row6_t[0:1, :]
            nc.scalar.copy(out=row6, in_=ovs[h6][D:D + 1, :])
            sd6 = dramp.tile([N], F32, tag="sd", name="sd6")
            nc.sync.dma_start(out=sd6, in_=row6)
            s6c = small.tile([P, NT], F32, tag="s_sp", name="s6c")
            nc.sync.dma_start(
                out=s6c, in_=sd6.rearrange("(k p) -> p k", p=P))
            av_head(h7, JT - 2)
            av_head(h7, JT - 1)
            row7_t = small.tile([D, N], F32, tag="rep", name="row7")
            row7 = row7_t[0:1, :]
            nc.scalar.copy(out=row7, in_=ovs[h7][D:D + 1, :])
            sd7 = dramp.tile([N], F32, tag="sd", name="sd7")
            nc.sync.dma_start(out=sd7, in_=row7)
            s7c = small.tile([P, NT], F32, tag="s_sp", name="s7c")
            nc.sync.dma_start(
                out=s7c, in_=sd7.rearrange("(k p) -> p k", p=P))

            for it in range(2, NT):
                a_group(it)

            # unnormalized O' for both heads (lhsT of the B matmuls)
            oT3u = persist.tile([P, N], BF16)
            ov6 = ovs.pop(h6)
            nc.vector.tensor_copy(out=oT3u[0:D, :], in_=ov6[0:D, :])
            ov7 = ovs.pop(h7)
            nc.vector.tensor_copy(out=oT3u[D:P, :], in_=ov7[0:D, :])
            r6c = small.tile([P, NT], F32, tag="r_sp", name="r6c")
            nc.vector.reciprocal(r6c, s6c)
            r7c = small.tile([P, NT], F32, tag="r_sp", name="r7c")
            nc.vector.reciprocal(r7c, s7c)
            del ets[h6], ets[h7]

            # per output tile: two K=64 head-partials (alternating PE row
            # tiles), Scalar applies 1/denom per partition, DVE folds in
            # the m=0..2+bias partial
            for it in range(NT):
                pb = psS.tile([P, N], F32, tag="st", name=f"pb{it}")
                for half, hp in ((0, 0), (1, D)):
                    nc.tensor.matmul(
                        pb[:, half * 512:half * 512 + C],
                        lhsT=oT3u[hp:hp + D, it * P:(it + 1) * P],
                        rhs=wo_b[hp:hp + D, 3, :],
                        start=True,
                        stop=True,
                    )
                t6 = small.tile([P, C], F32, tag="t6", name=f"t6_{it}")
                nc.scalar.activation(
                    out=t6, in_=pb[:, 0:C],
                    func=mybir.ActivationFunctionType.Copy,
                    scale=r6c[:, it:it + 1])
                t67 = small.tile([P, C], F32, tag="t7", name=f"t67_{it}")
                nc.vector.scalar_tensor_tensor(
                    out=t67, in0=pb[:, 512:512 + C],
                    scalar=r7c[:, it:it + 1], in1=t6,
                    op0=mybir.AluOpType.mult, op1=mybir.AluOpType.add)
                fin = small.tile([P, C], F32, tag="fin", bufs=3)
                eng = nc.vector if it % 2 == 0 else nc.gpsimd
                eng.tensor_add(fin, t67, a_sb[:, it, :])
                nc.sync.dma_start(out=out_d[it * P:(it + 1) * P, :], in_=fin)

    return nc


def kernel(x, Wq, Wkv, Wo, bo):
    from concourse.bass_utils import run_bass_kernel_spmd

    nc = build_nc()
    nc.compile()
    x = np.asarray(x)
    xs = np.ascontiguousarray(x.reshape(B, C, N)).astype(np.float32, copy=False)
    in_maps = [
        {
            "x": xs[b],
            "Wq": np.asarray(Wq, dtype=np.float32),
            "Wkv": np.asarray(Wkv, dtype=np.float32),
            "Wo": np.asarray(Wo, dtype=np.float32),
            "bo": np.asarray(bo, dtype=np.float32),
        }
        for b in range(B)
    ]
    res = run_bass_kernel_spmd(nc, in_maps, list(range(B)))
    return np.stack([res.results[b]["out"] for b in range(B)], axis=0)
